# revision 1
# baseline (speedup 1.0000x reference)
"""Trainium2 Bass kernel for nn_MAMoE (conv-MoE -> row attention -> MLP-MoE).

Sharding: 8 cores = (batch b in 0..3) x (H-half in 0..1). All routing is
per-token; the reference's swapaxes(1,2) means attention row r produces
output column w=r, so each core independently computes the full pipeline
for its 48 attention rows and the host reassembles along W.

All large matmuls run as float32r (full-rate fp32 storage, ~1e-3 matmul
accuracy); small-N attention matmuls run fp32.
"""
import numpy as np

import concourse.bass as bass
import concourse.mybir as mybir
import concourse.tile as tile
from concourse import bacc
from concourse.bass_utils import run_bass_kernel_spmd
from concourse.masks import make_identity

F32 = mybir.dt.float32
F32R = mybir.dt.float32r

B, HH, WW, C = 4, 96, 96, 384
HD = 128
SCALE = float((HD // 3) ** -0.5)  # 42**-0.5
N_CORES = 8
R = 48            # attention rows per core
RP = 65           # padded rows (R + 2*8) + 1 slack row for shifted flat reads
SP = 128          # padded W (96 + 2*8, padded to 512B row stride)
T = R * 96        # tokens per core = 4608
NT = 512          # tokens per MLP tile
NTILES = T // NT  # 9
GROUPS = R // 4   # 12 groups of 4 rows
GN = 4 * 96       # tokens per group = 384

TAPS_A = [
    [(dr, ds) for dr in (-1, 0, 1) for ds in (-1, 0, 1)],
    [(dr, 0) for dr in range(-4, 5)],
    [(0, ds) for ds in range(-4, 5)],
]
TAPS_B = [
    [(dr, ds) for dr in (-2, 0, 2) for ds in (-2, 0, 2)],
    [(dr, 0) for dr in range(-8, 9, 2)],
    [(0, ds) for ds in range(-8, 9, 2)],
]

_CACHED_NC = None


def build_kernel():
    nc = bacc.Bacc("TRN2", target_bir_lowering=False, debug=False)

    xp = nc.dram_tensor("xp", [C, RP, SP], F32R, kind="ExternalInput").ap()
    wca = nc.dram_tensor("wca", [3, 9, HD, HD], F32R, kind="ExternalInput").ap()
    wcb = nc.dram_tensor("wcb", [3, 9, HD, HD], F32R, kind="ExternalInput").ap()
    bca = nc.dram_tensor("bca", [HD, 3], F32, kind="ExternalInput").ap()
    bcb = nc.dram_tensor("bcb", [HD, 3], F32, kind="ExternalInput").ap()
    wgd = nc.dram_tensor("wgd", [3, HD, HD], F32R, kind="ExternalInput").ap()
    eb3 = nc.dram_tensor("eb3", [3, 384], F32R, kind="ExternalInput").ap()
    onesd = nc.dram_tensor("onesd", [1, HD], F32R, kind="ExternalInput").ap()
    wqk = nc.dram_tensor("wqk", [3, HD, 256], F32R, kind="ExternalInput").ap()
    wv = nc.dram_tensor("wv", [3, HD, HD], F32R, kind="ExternalInput").ap()
    bap = nc.dram_tensor("bap", [HD, 3], F32, kind="ExternalInput").ap()
    wgf = nc.dram_tensor("wgf", [3, HD, HD], F32R, kind="ExternalInput").ap()
    w1 = nc.dram_tensor("w1", [3, 3, HD, 1536], F32R, kind="ExternalInput").ap()
    b1 = nc.dram_tensor("b1", [HD, 3, 12], F32, kind="ExternalInput").ap()
    w2 = nc.dram_tensor("w2", [3, 12, HD, C], F32R, kind="ExternalInput").ap()
    b2r = nc.dram_tensor("b2r", [3, C], F32R, kind="ExternalInput").ap()
    bpr = nc.dram_tensor("bpr", [HD, 3], F32, kind="ExternalInput").ap()
    out_cm = nc.dram_tensor("out_cm", [C, T], F32, kind="ExternalOutput").ap()

    with tile.TileContext(nc) as tc:
        with tc.tile_pool(name="consts", bufs=1) as consts, \
             tc.tile_pool(name="persist", bufs=1) as persist:
            ones_r = consts.tile([1, HD], F32R)
            nc.sync.dma_start(out=ones_r, in_=onesd)
            ident = consts.tile([HD, HD], F32)
            make_identity(nc, ident)
            identb = consts.tile([HD, HD], mybir.dt.bfloat16)
            nc.vector.tensor_copy(identb, ident)

            bca_sb = persist.tile([HD, 3], F32)
            nc.sync.dma_start(out=bca_sb, in_=bca)
            bcb_sb = persist.tile([HD, 3], F32)
            nc.sync.dma_start(out=bcb_sb, in_=bcb)
            bap_sb = persist.tile([HD, 3], F32)
            nc.sync.dma_start(out=bap_sb, in_=bap)

            xc_t = [persist.tile([HD, T], F32R, tag=f"xc{i}", name=f"xc{i}") for i in range(3)]

            # ---------------- Phase A: conv MoE + attention per branch ----
            with tc.tile_pool(name="xpool", bufs=2) as xpool, \
                 tc.tile_pool(name="wpoolA", bufs=2) as wpoolA, \
                 tc.tile_pool(name="gpool", bufs=2) as gpool, \
                 tc.tile_pool(name="apool", bufs=3) as apool, \
                 tc.tile_pool(name="psC", bufs=3, space="PSUM") as psC, \
                 tc.tile_pool(name="psT", bufs=5, space="PSUM") as psT:
                for i in range(3):
                    xp_sb = xpool.tile([HD, RP, SP], F32R, tag="xp")
                    nc.sync.dma_start(out=xp_sb[:, :24, :],
                                      in_=xp[i * HD:(i + 1) * HD, :24, :])
                    nc.sync.dma_start(out=xp_sb[:, 24:, :],
                                      in_=xp[i * HD:(i + 1) * HD, 24:, :])
                    wca_sb = wpoolA.tile([HD, 9, HD], F32R, tag="wca")
                    nc.sync.dma_start(out=wca_sb[:, :3, :],
                                      in_=wca[i, :3].rearrange("a p b -> p a b"))
                    nc.sync.dma_start(out=wca_sb[:, 3:, :],
                                      in_=wca[i, 3:].rearrange("a p b -> p a b"))
                    wcb_sb = wpoolA.tile([HD, 9, HD], F32R, tag="wcb")
                    nc.sync.dma_start(out=wcb_sb[:, :3, :],
                                      in_=wcb[i, :3].rearrange("a p b -> p a b"))
                    nc.sync.dma_start(out=wcb_sb[:, 3:, :],
                                      in_=wcb[i, 3:].rearrange("a p b -> p a b"))
                    wgd_sb = wpoolA.tile([HD, HD], F32R, tag="wgd")
                    nc.sync.dma_start(out=wgd_sb, in_=wgd[i])
                    wqk_sb = wpoolA.tile([HD, 256], F32R, tag="wqk")
                    nc.sync.dma_start(out=wqk_sb, in_=wqk[i])
                    wv_sb = wpoolA.tile([HD, HD], F32R, tag="wv")
                    nc.sync.dma_start(out=wv_sb, in_=wv[i])

                    xpf = xp_sb.rearrange("p r s -> p (r s)")
                    for g in range(GROUPS):
                        rb = 8 + 4 * g  # xp row of first moe row in group
                        fo = rb * SP    # flat offset of group start
                        # gate logits first so ACT/DVE gate math hides
                        # under the conv matmuls
                        plg = psC.tile([HD, NT], F32, tag="ps")
                        nc.tensor.matmul(plg, wgd_sb, xpf[:, fo:fo + NT],
                                         start=True, stop=True)
                        ex = gpool.tile([HD, NT], F32, tag="ex")
                        nc.scalar.activation(ex, plg,
                                             mybir.ActivationFunctionType.Tanh,
                                             scale=-0.5)
                        # two expert convs over full padded rows (N=512,
                        # contiguous; shifted tails land in pad columns)
                        pa = psC.tile([HD, NT], F32, tag="ps")
                        for ti, (dr, ds) in enumerate(TAPS_A[i]):
                            nc.tensor.matmul(
                                pa, wca_sb[:, ti, :],
                                xpf[:, fo + dr * SP + ds: fo + dr * SP + ds + NT],
                                start=(ti == 0), stop=(ti == 8))
                        pb = psC.tile([HD, NT], F32, tag="ps")
                        for ti, (dr, ds) in enumerate(TAPS_B[i]):
                            nc.tensor.matmul(
                                pb, wcb_sb[:, ti, :],
                                xpf[:, fo + dr * SP + ds: fo + dr * SP + ds + NT],
                                start=(ti == 0), stop=(ti == 8))

                        # moe = g0*(ca - cb) + cb  (bias-add fused on ACT)
                        ca = gpool.tile([HD, NT], F32, tag="ca")
                        nc.scalar.activation(ca, pa,
                                             mybir.ActivationFunctionType.Identity,
                                             bias=bca_sb[:, i:i + 1], scale=0.5)
                        cb = gpool.tile([HD, NT], F32, tag="cb")
                        nc.scalar.activation(cb, pb,
                                             mybir.ActivationFunctionType.Identity,
                                             bias=bcb_sb[:, i:i + 1], scale=0.5)
                        dd = gpool.tile([HD, NT], F32, tag="dd")
                        nc.vector.tensor_sub(dd, ca, cb)
                        d2 = gpool.tile([HD, NT], F32, tag="d2")
                        nc.vector.tensor_mul(d2, dd, ex)
                        ss = gpool.tile([HD, NT], F32, tag="ss")
                        nc.vector.tensor_add(ss, ca, cb)
                        moe = gpool.tile([HD, NT], F32R, tag="moe")
                        nc.vector.tensor_add(moe, ss, d2)
                        # q, k projections (channel-major, full rows)
                        pq = psT.tile([HD, NT], F32, tag="ps")
                        nc.tensor.matmul(pq, wqk_sb[:, 0:HD], moe,
                                         start=True, stop=True)
                        q_sb = gpool.tile([HD, NT], mybir.dt.bfloat16, tag="q")
                        nc.scalar.copy(q_sb, pq)
                        pk = psT.tile([HD, NT], F32, tag="ps")
                        nc.tensor.matmul(pk, wqk_sb[:, HD:256], moe,
                                         start=True, stop=True)
                        k_sb = gpool.tile([HD, NT], mybir.dt.bfloat16, tag="k")
                        nc.scalar.copy(k_sb, pk)
                        # vT per row: [96 tok, 128 ch]
                        pvt = psT.tile([96, 4 * HD], F32, tag="ps")
                        for j in range(4):
                            nc.tensor.matmul(pvt[:, j * HD:(j + 1) * HD],
                                             moe[:, j * SP + 8: j * SP + 104],
                                             wv_sb, start=True, stop=True)
                        vt_sb = apool.tile([96, 4 * HD], mybir.dt.bfloat16, tag="vt")
                        nc.vector.tensor_copy(vt_sb, pvt)
                        # scores + softmax (no max-sub: logits are tiny)
                        psc = psT.tile([96, GN], F32, tag="ps")
                        for j in range(4):
                            nc.tensor.matmul(psc[:, j * 96:(j + 1) * 96],
                                             q_sb[:, j * SP + 8: j * SP + 104],
                                             k_sb[:, j * SP + 8: j * SP + 104],
                                             start=True, stop=True)
                        probs = apool.tile([96, GN], mybir.dt.bfloat16, tag="probs")
                        nc.scalar.activation(probs, psc,
                                             mybir.ActivationFunctionType.Exp,
                                             scale=SCALE)
                        zsum = apool.tile([96, 4], F32, tag="zsum")
                        nc.vector.tensor_reduce(
                            zsum, probs.rearrange("p (j q) -> p j q", q=96),
                            axis=mybir.AxisListType.X, op=mybir.AluOpType.add)
                        rec = apool.tile([96, 4], F32, tag="rec")
                        nc.vector.reciprocal(rec, zsum)
                        pn = apool.tile([96, GN], mybir.dt.bfloat16, tag="pn")
                        for j in range(4):
                            nc.scalar.activation(
                                pn[:, j * 96:(j + 1) * 96],
                                probs[:, j * 96:(j + 1) * 96],
                                mybir.ActivationFunctionType.Copy,
                                scale=rec[:, j:j + 1])
                        ppt = psT.tile([96, GN], mybir.dt.bfloat16, tag="ps")
                        for j in range(4):
                            nc.tensor.transpose(ppt[:, j * 96:(j + 1) * 96],
                                                pn[:, j * 96:(j + 1) * 96],
                                                identb[:96, :96])
                        pt_sb = apool.tile([96, GN], mybir.dt.bfloat16, tag="pt")
                        nc.vector.tensor_copy(pt_sb, ppt)
                        po = psT.tile([HD, GN], F32, tag="ps")
                        for j in range(4):
                            nc.tensor.matmul(po[:, j * 96:(j + 1) * 96],
                                             vt_sb[:, j * HD:(j + 1) * HD],
                                             pt_sb[:, j * 96:(j + 1) * 96],
                                             start=True, stop=True)
                        nc.scalar.activation(
                            xc_t[i][:, g * GN:(g + 1) * GN], po,
                            mybir.ActivationFunctionType.Identity,
                            bias=bap_sb[:, i:i + 1])

            # ---------------- Phase B: final MLP MoE + proj ---------------
            with tc.tile_pool(name="wpoolB", bufs=1) as wpoolB, \
                 tc.tile_pool(name="bpool", bufs=3) as bpool, \
                 tc.tile_pool(name="spool", bufs=1) as spool, \
                 tc.tile_pool(name="gpoolB", bufs=2) as gpoolB, \
                 tc.tile_pool(name="psL", bufs=3, space="PSUM") as psL, \
                 tc.tile_pool(name="psGB", bufs=1, space="PSUM") as psGB, \
                 tc.tile_pool(name="psPG", bufs=1, space="PSUM") as psPG, \
                 tc.tile_pool(name="psB", bufs=3, space="PSUM") as psB:
                b1_sb = wpoolB.tile([HD, 3, 12], F32)
                nc.sync.dma_start(out=b1_sb, in_=b1)
                b2r_sb = wpoolB.tile([3, C], F32R)
                nc.sync.dma_start(out=b2r_sb, in_=b2r)
                wgf_sb = wpoolB.tile([HD, 3, HD], F32R)
                nc.sync.dma_start(out=wgf_sb, in_=wgf.rearrange("a p b -> p a b"))
                bpr_sb = wpoolB.tile([HD, 3], F32)
                nc.sync.dma_start(out=bpr_sb, in_=bpr)
                eb3_sb = wpoolB.tile([3, 384], F32R)
                nc.sync.dma_start(out=eb3_sb, in_=eb3)
                w1_sb = []
                w2_sb = []
                for e in range(3):
                    t1 = wpoolB.tile([HD, 3, 1536], F32R, tag=f"w1_{e}", name=f"w1_{e}")
                    nc.sync.dma_start(out=t1, in_=w1[e].rearrange("a p b -> p a b"))
                    w1_sb.append(t1)
                    t2 = wpoolB.tile([HD, 12, C], F32R, tag=f"w2_{e}", name=f"w2_{e}")
                    nc.sync.dma_start(out=t2, in_=w2[e].rearrange("a p b -> p a b"))
                    w2_sb.append(t2)

                def gating_part1a(t):
                    """logits matmul for tile t."""
                    t0 = t * NT
                    plg = psGB.tile([HD, NT], F32, tag="ps", name="plg")
                    for kc in range(3):
                        nc.tensor.matmul(plg, wgf_sb[:, kc, :],
                                         xc_t[kc][:, t0:t0 + NT],
                                         start=(kc == 0), stop=(kc == 2))
                    lsb = gpoolB.tile([3, NT], F32, tag="lsb", name="lsb")
                    nc.vector.tensor_copy(lsb, plg[0:3, :])
                    return lsb

                def gating_part1b(lsb):
                    """token-major top-2 softmax math."""
                    plt = psGB.tile([HD, 12], F32, tag="ps", name="plt")
                    for t4 in range(4):
                        nc.tensor.transpose(plt[:, t4 * 3:(t4 + 1) * 3],
                                            lsb[:, t4 * HD:(t4 + 1) * HD],
                                            ident[:3, :3])
                    lt = gpoolB.tile([HD, 12], F32, tag="lt", name="lt")
                    nc.vector.tensor_copy(lt, plt)
                    l3 = lt.rearrange("p (j e) -> p j e", e=3)
                    mx = gpoolB.tile([HD, 4], F32, tag="mx", name="mx")
                    nc.vector.tensor_reduce(mx, l3, axis=mybir.AxisListType.X,
                                            op=mybir.AluOpType.max)
                    mn = gpoolB.tile([HD, 4], F32, tag="mn", name="mn")
                    nc.vector.tensor_reduce(mn, l3, axis=mybir.AxisListType.X,
                                            op=mybir.AluOpType.min)
                    sm = gpoolB.tile([HD, 4], F32, tag="sm", name="sm")
                    nc.vector.tensor_reduce(sm, l3, axis=mybir.AxisListType.X,
                                            op=mybir.AluOpType.add)
                    t1 = gpoolB.tile([HD, 4], F32, tag="t1", name="t1")
                    nc.vector.tensor_sub(t1, sm, mx)
                    mid = gpoolB.tile([HD, 4], F32, tag="mid", name="mid")
                    nc.vector.tensor_sub(mid, t1, mn)
                    dm = gpoolB.tile([HD, 4], F32, tag="dm", name="dm")
                    nc.vector.tensor_sub(dm, mx, mid)
                    th = gpoolB.tile([HD, 4], F32, tag="th", name="th")
                    nc.scalar.activation(th, dm,
                                         mybir.ActivationFunctionType.Tanh,
                                         scale=0.5)
                    gmx = gpoolB.tile([HD, 4], F32, tag="gmx", name="gmx")
                    nc.vector.tensor_scalar(gmx, th, 0.5, 0.5,
                                            op0=mybir.AluOpType.mult,
                                            op1=mybir.AluOpType.add)
                    eqx = gpoolB.tile([HD, 12], F32, tag="eqx", name="eqx")
                    eqn = gpoolB.tile([HD, 12], F32, tag="eqn", name="eqn")
                    for t4 in range(4):
                        sl = slice(t4 * 3, (t4 + 1) * 3)
                        nc.vector.tensor_scalar(eqx[:, sl], lt[:, sl],
                                                mx[:, t4:t4 + 1], None,
                                                op0=mybir.AluOpType.is_equal)
                        nc.vector.tensor_scalar(eqn[:, sl], lt[:, sl],
                                                mn[:, t4:t4 + 1], None,
                                                op0=mybir.AluOpType.is_equal)
                    # u = 1 - eqx - eqn (mid indicator); g = gmx*(eqx-u) + u
                    s1 = gpoolB.tile([HD, 12], F32, tag="s1", name="s1")
                    nc.vector.tensor_add(s1, eqx, eqn)
                    u = gpoolB.tile([HD, 12], F32, tag="u", name="u")
                    nc.vector.tensor_scalar(u, s1, -1.0, 1.0,
                                            op0=mybir.AluOpType.mult,
                                            op1=mybir.AluOpType.add)
                    d0 = gpoolB.tile([HD, 12], F32, tag="d0", name="d0")
                    nc.vector.tensor_sub(d0, eqx, u)
                    p0 = gpoolB.tile([HD, 12], F32, tag="p0", name="p0")
                    for t4 in range(4):
                        sl = slice(t4 * 3, (t4 + 1) * 3)
                        nc.vector.tensor_scalar_mul(p0[:, sl], d0[:, sl],
                                                    gmx[:, t4:t4 + 1])
                    gm = gpoolB.tile([HD, 12], F32, tag="gm", name="gm")
                    nc.vector.tensor_add(gm, p0, u)
                    return gm

                def gating_part2(gm):
                    """expert-major gates [3, NT] from token-major gm."""
                    pgt = psGB.tile([3, NT], F32, tag="ps", name="pgt")
                    for t4 in range(4):
                        nc.tensor.transpose(pgt[:, t4 * HD:(t4 + 1) * HD],
                                            gm[:, t4 * 3:(t4 + 1) * 3],
                                            ident)
                    gates_r = gpoolB.tile([3, NT], F32R, tag="gates",
                                          name="gates_r")
                    nc.scalar.copy(gates_r, pgt)
                    return gates_r

                gm_next = gating_part1b(gating_part1a(0))
                for t in range(NTILES):
                    t0 = t * NT
                    gates_r = gating_part2(gm_next)
                    lsb_next = gating_part1a(t + 1) if t + 1 < NTILES else None

                    pd = [psL.tile([HD, NT], F32, tag="down", name=f"pd{_i}") for _i in range(3)]
                    for e in range(3):
                        if e == 1 and lsb_next is not None:
                            gm_next = gating_part1b(lsb_next)
                        pgb = psPG.tile([HD, NT], F32, tag="pgb", name="pgb")
                        nc.tensor.matmul(pgb, eb3_sb[:, e * HD:(e + 1) * HD],
                                         gates_r, start=True, stop=True)
                        for m in range(12):
                            pu = psB.tile([HD, NT], F32, tag="ps", name="pu")
                            for kc in range(3):
                                nc.tensor.matmul(
                                    pu, w1_sb[e][:, kc, m * HD:(m + 1) * HD],
                                    xc_t[kc][:, t0:t0 + NT],
                                    start=(kc == 0), stop=(kc == 2))
                            h = bpool.tile([HD, NT], F32, tag="h")
                            nc.scalar.activation(
                                h, pu, mybir.ActivationFunctionType.Gelu,
                                bias=b1_sb[:, e, m:m + 1])
                            hs = bpool.tile([HD, NT], F32R, tag="hs")
                            nc.vector.tensor_mul(hs, h, pgb)
                            for mp in range(3):
                                nc.tensor.matmul(
                                    pd[mp], w2_sb[e][:, m, mp * HD:(mp + 1) * HD],
                                    hs, start=(e == 0 and m == 0), stop=False)
                    for mp in range(3):
                        nc.tensor.matmul(pd[mp], b2r_sb[:, mp * HD:(mp + 1) * HD],
                                         gates_r, start=False, stop=True)
                    for mp in range(3):
                        osb = bpool.tile([HD, NT], F32, tag="osb")
                        nc.scalar.activation(osb, pd[mp],
                                             mybir.ActivationFunctionType.Identity,
                                             bias=bpr_sb[:, mp:mp + 1])
                        nc.sync.dma_start(
                            out=out_cm[mp * HD:(mp + 1) * HD, t0:t0 + NT],
                            in_=osb)
    nc.compile()
    return nc


def _prep_inputs(x, w_e1, b_e1, w_e2, b_e2, w_e3, b_e3, w_e4, b_e4, w_e5, b_e5,
                 w_e6, b_e6, wg1, wg2, wg3, w_qkv, w_attn_proj, b_attn_proj,
                 wg_final, w_mlp1, b_mlp1, w_mlp2, b_mlp2, w_proj, b_proj):
    f = np.float32
    shared = {}
    shared["wca"] = np.ascontiguousarray(np.stack([
        w_e1.reshape(9, HD, HD), w_e3.reshape(9, HD, HD),
        w_e5.reshape(9, HD, HD)]), dtype=f)
    shared["wcb"] = np.ascontiguousarray(np.stack([
        w_e2.reshape(9, HD, HD), w_e4.reshape(9, HD, HD),
        w_e6.reshape(9, HD, HD)]), dtype=f)
    shared["bca"] = np.ascontiguousarray(
        np.stack([b_e1, b_e3, b_e5], axis=1) * 0.5, dtype=f)
    shared["bcb"] = np.ascontiguousarray(
        np.stack([b_e2, b_e4, b_e6], axis=1) * 0.5, dtype=f)
    wgs = np.stack([wg1, wg2, wg3])
    shared["wgd"] = np.ascontiguousarray(
        np.repeat((wgs[:, :, 1] - wgs[:, :, 0])[:, :, None], HD, axis=2),
        dtype=f)
    eb3 = np.zeros((3, 384), f)
    for e in range(3):
        eb3[e, e * 128:(e + 1) * 128] = 1.0
    shared["eb3"] = eb3
    shared["onesd"] = np.ones((1, 128), f)
    shared["wqk"] = np.ascontiguousarray(w_qkv[:, :, :256], dtype=f)
    wv64 = np.asarray(w_qkv[:, :, 256:], dtype=np.float64)
    wap64 = np.asarray(w_attn_proj, dtype=np.float64)
    shared["wv"] = np.ascontiguousarray(
        np.einsum("ick,iko->ico", wv64, wap64), dtype=f)
    shared["bap"] = np.ascontiguousarray(b_attn_proj.T, dtype=f)
    shared["wgf"] = np.ascontiguousarray(
        np.tile(wg_final.reshape(3, HD, 3), (1, 1, 43))[:, :, :HD], dtype=f)
    shared["w1"] = np.ascontiguousarray(w_mlp1.reshape(3, 3, HD, 1536), dtype=f)
    shared["b1"] = np.ascontiguousarray(
        b_mlp1.reshape(3, 12, HD).transpose(2, 0, 1), dtype=f)
    w2p = np.asarray(w_mlp2, dtype=np.float64) @ np.asarray(w_proj, np.float64)
    shared["w2"] = np.ascontiguousarray(w2p.reshape(3, 12, HD, C), dtype=f)
    shared["b2r"] = np.ascontiguousarray(
        np.asarray(b_mlp2, np.float64) @ np.asarray(w_proj, np.float64), dtype=f)
    shared["bpr"] = np.ascontiguousarray(b_proj.reshape(3, HD).T, dtype=f)

    in_maps = []
    for c in range(N_CORES):
        b, half = c // 2, c % 2
        r0 = half * R
        slab = np.zeros((C, RP, SP), f)
        glo, ghi = max(0, r0 - 8), min(HH, r0 + R + 8)
        plo = glo - (r0 - 8)
        slab[:, plo:plo + (ghi - glo), 8:104] = \
            np.asarray(x[b, glo:ghi], dtype=f).transpose(2, 0, 1)
        m = dict(shared)
        m["xp"] = np.ascontiguousarray(slab)
        in_maps.append(m)
    return in_maps


def kernel(**inputs):
    global _CACHED_NC
    if _CACHED_NC is None:
        _CACHED_NC = build_kernel()
    nc = _CACHED_NC
    in_maps = _prep_inputs(**{k: np.asarray(v) for k, v in inputs.items()})
    res = None
    for attempt in range(3):
        try:
            res = run_bass_kernel_spmd(nc, in_maps,
                                       core_ids=list(range(N_CORES)))
            break
        except Exception:
            if attempt == 2:
                raise
            import time
            time.sleep(2.0)
    out = np.empty((B, HH, WW, C), np.float32)
    for c in range(N_CORES):
        b, half = c // 2, c % 2
        slab = res.results[c]["out_cm"].reshape(C, R, 96)
        out[b, :, half * R:(half + 1) * R, :] = slab.transpose(2, 1, 0)
    return out



# revision 2
# speedup vs baseline: 1.0679x; 1.0679x over previous
"""Trainium2 Bass kernel for nn_MAMoE (conv-MoE -> row attention -> MLP-MoE).

Sharding: 8 cores = (batch b in 0..3) x (H-half in 0..1). All routing is
per-token; the reference's swapaxes(1,2) means attention row r produces
output column w=r, so each core independently computes the full pipeline
for its 48 attention rows and the host reassembles along W.

Layout: padded row stride 104 (8 zero cols serve as both right halo of
row r and left halo of row r+1). bf16 inputs/weights with fp32 PSUM
accumulation; Phase-B weights preloaded during Phase A.
"""
import numpy as np
import ml_dtypes

import concourse.bass as bass
import concourse.mybir as mybir
import concourse.tile as tile
from concourse import bacc
from concourse.bass_utils import run_bass_kernel_spmd
from concourse.masks import make_identity

F32 = mybir.dt.float32
F32R = mybir.dt.float32r
BF16 = mybir.dt.bfloat16
BF = ml_dtypes.bfloat16

B, HH, WW, C = 4, 96, 96, 384
HD = 128
SCALE = float((HD // 3) ** -0.5)  # 42**-0.5
N_CORES = 8
R = 48            # attention rows per core
RP = 66           # slack row + 8 halo + 48 + 8 halo + 1 slack row
SP = 104          # padded row stride (8 zero pad + 96 valid)
T = R * 96        # tokens per core = 4608
NT = 512          # tokens per MLP tile
NTILES = T // NT  # 9
GROUPS = R // 4   # 12 groups of 4 rows
GN = 4 * 96       # attention tokens per group = 384
CN = 4 * SP       # conv tokens per group (with pads) = 416

TAPS_A = [
    [(dr, ds) for dr in (-1, 0, 1) for ds in (-1, 0, 1)],
    [(dr, 0) for dr in range(-4, 5)],
    [(0, ds) for ds in range(-4, 5)],
]
TAPS_B = [
    [(dr, ds) for dr in (-2, 0, 2) for ds in (-2, 0, 2)],
    [(dr, 0) for dr in range(-8, 9, 2)],
    [(0, ds) for ds in range(-8, 9, 2)],
]

_CACHED_NC = None


def build_kernel():
    nc = bacc.Bacc("TRN2", target_bir_lowering=False, debug=False)

    xp = nc.dram_tensor("xp", [C, RP, SP], BF16, kind="ExternalInput").ap()
    wca = nc.dram_tensor("wca", [3, HD, 9, HD], BF16, kind="ExternalInput").ap()
    wcb = nc.dram_tensor("wcb", [3, HD, 9, HD], BF16, kind="ExternalInput").ap()
    bca = nc.dram_tensor("bca", [HD, 3], F32, kind="ExternalInput").ap()
    bcb = nc.dram_tensor("bcb", [HD, 3], F32, kind="ExternalInput").ap()
    wgd = nc.dram_tensor("wgd", [3, HD, HD], BF16, kind="ExternalInput").ap()
    eb3 = nc.dram_tensor("eb3", [3, 384], F32R, kind="ExternalInput").ap()
    wqk = nc.dram_tensor("wqk", [3, HD, 256], BF16, kind="ExternalInput").ap()
    wv = nc.dram_tensor("wv", [3, HD, HD], BF16, kind="ExternalInput").ap()
    bap = nc.dram_tensor("bap", [HD, 3], F32, kind="ExternalInput").ap()
    wgf = nc.dram_tensor("wgf", [3, HD, HD], BF16, kind="ExternalInput").ap()
    w1 = nc.dram_tensor("w1", [3, HD, 3, 1536], BF16, kind="ExternalInput").ap()
    b1 = nc.dram_tensor("b1", [HD, 3, 12], F32, kind="ExternalInput").ap()
    w2 = nc.dram_tensor("w2", [3, HD, 12, C], BF16, kind="ExternalInput").ap()
    b2r = nc.dram_tensor("b2r", [3, C], F32R, kind="ExternalInput").ap()
    bpr = nc.dram_tensor("bpr", [HD, 3], F32, kind="ExternalInput").ap()
    out_cm = nc.dram_tensor("out_cm", [C, T], F32, kind="ExternalOutput").ap()

    with tile.TileContext(nc) as tc:
        with tc.tile_pool(name="consts", bufs=1) as consts, \
             tc.tile_pool(name="persist", bufs=1) as persist:
            ident = consts.tile([HD, HD], F32)
            make_identity(nc, ident)
            identb = consts.tile([HD, HD], BF16)
            nc.vector.tensor_copy(identb, ident)

            bca_sb = persist.tile([HD, 3], F32)
            nc.sync.dma_start(out=bca_sb, in_=bca)
            bcb_sb = persist.tile([HD, 3], F32)
            nc.sync.dma_start(out=bcb_sb, in_=bcb)
            bap_sb = persist.tile([HD, 3], F32)
            nc.sync.dma_start(out=bap_sb, in_=bap)

            xc_t = [persist.tile([HD, T], BF16, tag=f"xc{i}", name=f"xc{i}") for i in range(3)]

            # Phase-B weights: preload during Phase A (contiguous DMA).
            b1_sb = persist.tile([HD, 3, 12], F32)
            b2r_sb = persist.tile([3, C], F32R)
            wgf_sb = persist.tile([HD, 3, HD], BF16)
            bpr_sb = persist.tile([HD, 3], F32)
            eb3_sb = persist.tile([3, 384], F32R)
            w1_sb = [persist.tile([HD, 3, 1536], BF16, tag=f"w1_{e}", name=f"w1_{e}")
                     for e in range(3)]
            w2_sb = [persist.tile([HD, 12, C], BF16, tag=f"w2_{e}", name=f"w2_{e}")
                     for e in range(3)]

            def load_phase_b_weights():
                nc.sync.dma_start(out=b1_sb, in_=b1)
                nc.sync.dma_start(out=b2r_sb, in_=b2r)
                nc.sync.dma_start(out=wgf_sb, in_=wgf.rearrange("a p b -> p a b"))
                nc.sync.dma_start(out=bpr_sb, in_=bpr)
                nc.sync.dma_start(out=eb3_sb, in_=eb3)
                for e in range(3):
                    nc.sync.dma_start(out=w1_sb[e], in_=w1[e])
                    nc.sync.dma_start(out=w2_sb[e], in_=w2[e])

            # ---------------- Phase A: conv MoE + attention per branch ----
            with tc.tile_pool(name="xpool", bufs=2) as xpool, \
                 tc.tile_pool(name="wpoolA", bufs=2) as wpoolA, \
                 tc.tile_pool(name="gpool", bufs=2) as gpool, \
                 tc.tile_pool(name="apool", bufs=3) as apool, \
                 tc.tile_pool(name="psC", bufs=3, space="PSUM") as psC, \
                 tc.tile_pool(name="psT", bufs=5, space="PSUM") as psT:
                for i in range(3):
                    xp_sb = xpool.tile([HD, RP, SP], BF16, tag="xp")
                    nc.sync.dma_start(out=xp_sb[:, :33, :],
                                      in_=xp[i * HD:(i + 1) * HD, :33, :])
                    wgd_sb = wpoolA.tile([HD, HD], BF16, tag="wgd")
                    nc.sync.dma_start(out=wgd_sb, in_=wgd[i])
                    wca_sb = wpoolA.tile([HD, 9, HD], BF16, tag="wca")
                    nc.sync.dma_start(out=wca_sb, in_=wca[i])
                    wcb_sb = wpoolA.tile([HD, 9, HD], BF16, tag="wcb")
                    nc.sync.dma_start(out=wcb_sb, in_=wcb[i])
                    wqk_sb = wpoolA.tile([HD, 256], BF16, tag="wqk")
                    nc.sync.dma_start(out=wqk_sb, in_=wqk[i])
                    wv_sb = wpoolA.tile([HD, HD], BF16, tag="wv")
                    nc.sync.dma_start(out=wv_sb, in_=wv[i])
                    nc.sync.dma_start(out=xp_sb[:, 33:, :],
                                      in_=xp[i * HD:(i + 1) * HD, 33:, :])
                    if i == 0:
                        load_phase_b_weights()

                    xpf = xp_sb.rearrange("p r s -> p (r s)")
                    for g in range(GROUPS):
                        rb = 9 + 4 * g  # slab row of first moe row in group
                        fo = rb * SP    # flat offset of group start
                        # gate logits first so ACT/DVE gate math hides
                        # under the conv matmuls
                        plg = psC.tile([HD, CN], F32, tag="ps")
                        nc.tensor.matmul(plg, wgd_sb, xpf[:, fo:fo + CN],
                                         start=True, stop=True)
                        ex = gpool.tile([HD, CN], F32, tag="ex")
                        nc.scalar.activation(ex, plg,
                                             mybir.ActivationFunctionType.Tanh,
                                             scale=-0.5)
                        # two expert convs over full padded rows (N=416,
                        # contiguous; shifted tails land in pad columns)
                        pa = psC.tile([HD, CN], F32, tag="ps")
                        for ti, (dr, ds) in enumerate(TAPS_A[i]):
                            nc.tensor.matmul(
                                pa, wca_sb[:, ti, :],
                                xpf[:, fo + dr * SP + ds: fo + dr * SP + ds + CN],
                                start=(ti == 0), stop=(ti == 8))
                        pb = psC.tile([HD, CN], F32, tag="ps")
                        for ti, (dr, ds) in enumerate(TAPS_B[i]):
                            nc.tensor.matmul(
                                pb, wcb_sb[:, ti, :],
                                xpf[:, fo + dr * SP + ds: fo + dr * SP + ds + CN],
                                start=(ti == 0), stop=(ti == 8))

                        # moe = g0*(ca - cb) + cb  (bias-add fused on ACT)
                        ca = gpool.tile([HD, CN], F32, tag="ca")
                        nc.scalar.activation(ca, pa,
                                             mybir.ActivationFunctionType.Identity,
                                             bias=bca_sb[:, i:i + 1], scale=0.5)
                        cb = gpool.tile([HD, CN], F32, tag="cb")
                        nc.scalar.activation(cb, pb,
                                             mybir.ActivationFunctionType.Identity,
                                             bias=bcb_sb[:, i:i + 1], scale=0.5)
                        dd = gpool.tile([HD, CN], F32, tag="dd")
                        nc.vector.tensor_sub(dd, ca, cb)
                        d2 = gpool.tile([HD, CN], F32, tag="d2")
                        nc.vector.tensor_mul(d2, dd, ex)
                        ss = gpool.tile([HD, CN], F32, tag="ss")
                        nc.vector.tensor_add(ss, ca, cb)
                        moe = gpool.tile([HD, CN], BF16, tag="moe")
                        nc.vector.tensor_add(moe, ss, d2)
                        # q, k projections (channel-major, full rows)
                        pq = psT.tile([HD, CN], F32, tag="ps")
                        nc.tensor.matmul(pq, wqk_sb[:, 0:HD], moe,
                                         start=True, stop=True)
                        q_sb = gpool.tile([HD, CN], BF16, tag="q")
                        nc.scalar.copy(q_sb, pq)
                        pk = psT.tile([HD, CN], F32, tag="ps")
                        nc.tensor.matmul(pk, wqk_sb[:, HD:256], moe,
                                         start=True, stop=True)
                        k_sb = gpool.tile([HD, CN], BF16, tag="k")
                        nc.scalar.copy(k_sb, pk)
                        # vT per row: [96 tok, 128 ch]
                        pvt = psT.tile([96, 4 * HD], F32, tag="ps")
                        for j in range(4):
                            nc.tensor.matmul(pvt[:, j * HD:(j + 1) * HD],
                                             moe[:, j * SP + 8: j * SP + SP],
                                             wv_sb, start=True, stop=True)
                        vt_sb = apool.tile([96, 4 * HD], BF16, tag="vt")
                        nc.vector.tensor_copy(vt_sb, pvt)
                        # scores + softmax (no max-sub: logits are tiny)
                        psc = psT.tile([96, GN], F32, tag="ps")
                        for j in range(4):
                            nc.tensor.matmul(psc[:, j * 96:(j + 1) * 96],
                                             q_sb[:, j * SP + 8: j * SP + SP],
                                             k_sb[:, j * SP + 8: j * SP + SP],
                                             start=True, stop=True)
                        probs = apool.tile([96, GN], BF16, tag="probs")
                        nc.scalar.activation(probs, psc,
                                             mybir.ActivationFunctionType.Exp,
                                             scale=SCALE)
                        zsum = apool.tile([96, 4], F32, tag="zsum")
                        nc.vector.tensor_reduce(
                            zsum, probs.rearrange("p (j q) -> p j q", q=96),
                            axis=mybir.AxisListType.X, op=mybir.AluOpType.add)
                        rec = apool.tile([96, 4], F32, tag="rec")
                        nc.vector.reciprocal(rec, zsum)
                        pn = apool.tile([96, GN], BF16, tag="pn")
                        for j in range(4):
                            nc.scalar.activation(
                                pn[:, j * 96:(j + 1) * 96],
                                probs[:, j * 96:(j + 1) * 96],
                                mybir.ActivationFunctionType.Copy,
                                scale=rec[:, j:j + 1])
                        ppt = psT.tile([96, GN], BF16, tag="ps")
                        for j in range(4):
                            nc.tensor.transpose(ppt[:, j * 96:(j + 1) * 96],
                                                pn[:, j * 96:(j + 1) * 96],
                                                identb[:96, :96])
                        pt_sb = apool.tile([96, GN], BF16, tag="pt")
                        nc.vector.tensor_copy(pt_sb, ppt)
                        po = psT.tile([HD, GN], F32, tag="ps")
                        for j in range(4):
                            nc.tensor.matmul(po[:, j * 96:(j + 1) * 96],
                                             vt_sb[:, j * HD:(j + 1) * HD],
                                             pt_sb[:, j * 96:(j + 1) * 96],
                                             start=True, stop=True)
                        nc.scalar.activation(
                            xc_t[i][:, g * GN:(g + 1) * GN], po,
                            mybir.ActivationFunctionType.Identity,
                            bias=bap_sb[:, i:i + 1])

            # ---------------- Phase B: final MLP MoE + proj ---------------
            with tc.tile_pool(name="bpool", bufs=3) as bpool, \
                 tc.tile_pool(name="gpoolB", bufs=2) as gpoolB, \
                 tc.tile_pool(name="psL", bufs=3, space="PSUM") as psL, \
                 tc.tile_pool(name="psGB", bufs=1, space="PSUM") as psGB, \
                 tc.tile_pool(name="psPG", bufs=1, space="PSUM") as psPG, \
                 tc.tile_pool(name="psB", bufs=3, space="PSUM") as psB:

                def gating_part1a(t):
                    """logits matmul for tile t."""
                    t0 = t * NT
                    plg = psGB.tile([HD, NT], F32, tag="ps", name="plg")
                    for kc in range(3):
                        nc.tensor.matmul(plg, wgf_sb[:, kc, :],
                                         xc_t[kc][:, t0:t0 + NT],
                                         start=(kc == 0), stop=(kc == 2))
                    lsb = gpoolB.tile([3, NT], F32, tag="lsb", name="lsb")
                    nc.vector.tensor_copy(lsb, plg[0:3, :])
                    return lsb

                def gating_part1b(lsb):
                    """token-major top-2 softmax math."""
                    plt = psGB.tile([HD, 12], F32, tag="ps", name="plt")
                    for t4 in range(4):
                        nc.tensor.transpose(plt[:, t4 * 3:(t4 + 1) * 3],
                                            lsb[:, t4 * HD:(t4 + 1) * HD],
                                            ident[:3, :3])
                    lt = gpoolB.tile([HD, 12], F32, tag="lt", name="lt")
                    nc.vector.tensor_copy(lt, plt)
                    l3 = lt.rearrange("p (j e) -> p j e", e=3)
                    mx = gpoolB.tile([HD, 4], F32, tag="mx", name="mx")
                    nc.vector.tensor_reduce(mx, l3, axis=mybir.AxisListType.X,
                                            op=mybir.AluOpType.max)
                    mn = gpoolB.tile([HD, 4], F32, tag="mn", name="mn")
                    nc.vector.tensor_reduce(mn, l3, axis=mybir.AxisListType.X,
                                            op=mybir.AluOpType.min)
                    sm = gpoolB.tile([HD, 4], F32, tag="sm", name="sm")
                    nc.vector.tensor_reduce(sm, l3, axis=mybir.AxisListType.X,
                                            op=mybir.AluOpType.add)
                    t1 = gpoolB.tile([HD, 4], F32, tag="t1", name="t1")
                    nc.vector.tensor_sub(t1, sm, mx)
                    mid = gpoolB.tile([HD, 4], F32, tag="mid", name="mid")
                    nc.vector.tensor_sub(mid, t1, mn)
                    dm = gpoolB.tile([HD, 4], F32, tag="dm", name="dm")
                    nc.vector.tensor_sub(dm, mx, mid)
                    th = gpoolB.tile([HD, 4], F32, tag="th", name="th")
                    nc.scalar.activation(th, dm,
                                         mybir.ActivationFunctionType.Tanh,
                                         scale=0.5)
                    gmx = gpoolB.tile([HD, 4], F32, tag="gmx", name="gmx")
                    nc.vector.tensor_scalar(gmx, th, 0.5, 0.5,
                                            op0=mybir.AluOpType.mult,
                                            op1=mybir.AluOpType.add)
                    eqx = gpoolB.tile([HD, 12], F32, tag="eqx", name="eqx")
                    eqn = gpoolB.tile([HD, 12], F32, tag="eqn", name="eqn")
                    for t4 in range(4):
                        sl = slice(t4 * 3, (t4 + 1) * 3)
                        nc.vector.tensor_scalar(eqx[:, sl], lt[:, sl],
                                                mx[:, t4:t4 + 1], None,
                                                op0=mybir.AluOpType.is_equal)
                        nc.vector.tensor_scalar(eqn[:, sl], lt[:, sl],
                                                mn[:, t4:t4 + 1], None,
                                                op0=mybir.AluOpType.is_equal)
                    # u = 1 - eqx - eqn (mid indicator); g = gmx*(eqx-u) + u
                    s1 = gpoolB.tile([HD, 12], F32, tag="s1", name="s1")
                    nc.vector.tensor_add(s1, eqx, eqn)
                    u = gpoolB.tile([HD, 12], F32, tag="u", name="u")
                    nc.vector.tensor_scalar(u, s1, -1.0, 1.0,
                                            op0=mybir.AluOpType.mult,
                                            op1=mybir.AluOpType.add)
                    d0 = gpoolB.tile([HD, 12], F32, tag="d0", name="d0")
                    nc.vector.tensor_sub(d0, eqx, u)
                    p0 = gpoolB.tile([HD, 12], F32, tag="p0", name="p0")
                    for t4 in range(4):
                        sl = slice(t4 * 3, (t4 + 1) * 3)
                        nc.vector.tensor_scalar_mul(p0[:, sl], d0[:, sl],
                                                    gmx[:, t4:t4 + 1])
                    gm = gpoolB.tile([HD, 12], BF16, tag="gm", name="gm")
                    nc.vector.tensor_add(gm, p0, u)
                    return gm

                def gating_part2(gm):
                    """expert-major gates [3, NT] from token-major gm."""
                    pgt = psGB.tile([3, NT], BF16, tag="ps", name="pgt")
                    for t4 in range(4):
                        nc.tensor.transpose(pgt[:, t4 * HD:(t4 + 1) * HD],
                                            gm[:, t4 * 3:(t4 + 1) * 3],
                                            identb)
                    gates_r = gpoolB.tile([3, NT], F32R, tag="gates",
                                          name="gates_r")
                    nc.scalar.copy(gates_r, pgt)
                    return gates_r

                gm_next = gating_part1b(gating_part1a(0))
                for t in range(NTILES):
                    t0 = t * NT
                    gates_r = gating_part2(gm_next)
                    lsb_next = gating_part1a(t + 1) if t + 1 < NTILES else None

                    pd = [psL.tile([HD, NT], F32, tag="down", name=f"pd{_i}") for _i in range(3)]
                    for e in range(3):
                        if e == 1 and lsb_next is not None:
                            gm_next = gating_part1b(lsb_next)
                        pgb = psPG.tile([HD, NT], F32, tag="pgb", name="pgb")
                        nc.tensor.matmul(pgb, eb3_sb[:, e * HD:(e + 1) * HD],
                                         gates_r, start=True, stop=True)
                        for m in range(12):
                            pu = psB.tile([HD, NT], F32, tag="ps", name="pu")
                            for kc in range(3):
                                nc.tensor.matmul(
                                    pu, w1_sb[e][:, kc, m * HD:(m + 1) * HD],
                                    xc_t[kc][:, t0:t0 + NT],
                                    start=(kc == 0), stop=(kc == 2))
                            h = bpool.tile([HD, NT], F32, tag="h")
                            nc.scalar.activation(
                                h, pu, mybir.ActivationFunctionType.Gelu,
                                bias=b1_sb[:, e, m:m + 1])
                            hs = bpool.tile([HD, NT], BF16, tag="hs")
                            nc.vector.tensor_mul(hs, h, pgb)
                            for mp in range(3):
                                nc.tensor.matmul(
                                    pd[mp], w2_sb[e][:, m, mp * HD:(mp + 1) * HD],
                                    hs, start=(e == 0 and m == 0), stop=False)
                    for mp in range(3):
                        nc.tensor.matmul(pd[mp], b2r_sb[:, mp * HD:(mp + 1) * HD],
                                         gates_r, start=False, stop=True)
                    for mp in range(3):
                        osb = bpool.tile([HD, NT], F32, tag="osb")
                        nc.scalar.activation(osb, pd[mp],
                                             mybir.ActivationFunctionType.Identity,
                                             bias=bpr_sb[:, mp:mp + 1])
                        nc.sync.dma_start(
                            out=out_cm[mp * HD:(mp + 1) * HD, t0:t0 + NT],
                            in_=osb)
    nc.compile()
    return nc


def _prep_inputs(x, w_e1, b_e1, w_e2, b_e2, w_e3, b_e3, w_e4, b_e4, w_e5, b_e5,
                 w_e6, b_e6, wg1, wg2, wg3, w_qkv, w_attn_proj, b_attn_proj,
                 wg_final, w_mlp1, b_mlp1, w_mlp2, b_mlp2, w_proj, b_proj):
    f = np.float32
    shared = {}
    # conv weights pre-transposed to [cin(p), tap, cout] for contiguous DMA
    shared["wca"] = np.ascontiguousarray(np.stack([
        w_e1.reshape(9, HD, HD).transpose(1, 0, 2),
        w_e3.reshape(9, HD, HD).transpose(1, 0, 2),
        w_e5.reshape(9, HD, HD).transpose(1, 0, 2)]).astype(BF))
    shared["wcb"] = np.ascontiguousarray(np.stack([
        w_e2.reshape(9, HD, HD).transpose(1, 0, 2),
        w_e4.reshape(9, HD, HD).transpose(1, 0, 2),
        w_e6.reshape(9, HD, HD).transpose(1, 0, 2)]).astype(BF))
    shared["bca"] = np.ascontiguousarray(
        np.stack([b_e1, b_e3, b_e5], axis=1) * 0.5, dtype=f)
    shared["bcb"] = np.ascontiguousarray(
        np.stack([b_e2, b_e4, b_e6], axis=1) * 0.5, dtype=f)
    wgs = np.stack([wg1, wg2, wg3])
    shared["wgd"] = np.ascontiguousarray(
        np.repeat((wgs[:, :, 1] - wgs[:, :, 0])[:, :, None], HD, axis=2)
        .astype(BF))
    eb3 = np.zeros((3, 384), f)
    for e in range(3):
        eb3[e, e * 128:(e + 1) * 128] = 1.0
    shared["eb3"] = eb3
    shared["wqk"] = np.ascontiguousarray(np.asarray(w_qkv[:, :, :256]).astype(BF))
    wv64 = np.asarray(w_qkv[:, :, 256:], dtype=np.float64)
    wap64 = np.asarray(w_attn_proj, dtype=np.float64)
    shared["wv"] = np.ascontiguousarray(
        np.einsum("ick,iko->ico", wv64, wap64).astype(BF))
    shared["bap"] = np.ascontiguousarray(b_attn_proj.T, dtype=f)
    shared["wgf"] = np.ascontiguousarray(
        np.tile(wg_final.reshape(3, HD, 3), (1, 1, 43))[:, :, :HD].astype(BF))
    shared["w1"] = np.ascontiguousarray(
        w_mlp1.reshape(3, 3, HD, 1536).transpose(0, 2, 1, 3).astype(BF))
    shared["b1"] = np.ascontiguousarray(
        b_mlp1.reshape(3, 12, HD).transpose(2, 0, 1), dtype=f)
    w2p = np.asarray(w_mlp2, dtype=np.float64) @ np.asarray(w_proj, np.float64)
    shared["w2"] = np.ascontiguousarray(
        w2p.reshape(3, 12, HD, C).transpose(0, 2, 1, 3).astype(BF))
    shared["b2r"] = np.ascontiguousarray(
        np.asarray(b_mlp2, np.float64) @ np.asarray(w_proj, np.float64), dtype=f)
    shared["bpr"] = np.ascontiguousarray(b_proj.reshape(3, HD).T, dtype=f)

    in_maps = []
    for c in range(N_CORES):
        b, half = c // 2, c % 2
        r0 = half * R
        slab = np.zeros((C, RP, SP), BF)
        glo, ghi = max(0, r0 - 8), min(HH, r0 + R + 8)
        plo = glo - (r0 - 8) + 1
        slab[:, plo:plo + (ghi - glo), 8:SP] = \
            np.asarray(x[b, glo:ghi]).astype(BF).transpose(2, 0, 1)
        m = dict(shared)
        m["xp"] = np.ascontiguousarray(slab)
        in_maps.append(m)
    return in_maps


def kernel(**inputs):
    global _CACHED_NC
    if _CACHED_NC is None:
        _CACHED_NC = build_kernel()
    nc = _CACHED_NC
    in_maps = _prep_inputs(**{k: np.asarray(v) for k, v in inputs.items()})
    res = None
    for attempt in range(3):
        try:
            res = run_bass_kernel_spmd(nc, in_maps,
                                       core_ids=list(range(N_CORES)))
            break
        except Exception:
            if attempt == 2:
                raise
            import time
            time.sleep(2.0)
    out = np.empty((B, HH, WW, C), np.float32)
    for c in range(N_CORES):
        b, half = c // 2, c % 2
        slab = res.results[c]["out_cm"].reshape(C, R, 96)
        out[b, :, half * R:(half + 1) * R, :] = slab.transpose(2, 1, 0)
    return out


# revision 3
# speedup vs baseline: 1.1050x; 1.0347x over previous
"""Trainium2 Bass kernel for nn_MAMoE (conv-MoE -> row attention -> MLP-MoE).

Sharding: 8 cores = (batch b in 0..3) x (H-half in 0..1). All routing is
per-token; the reference's swapaxes(1,2) means attention row r produces
output column w=r, so each core independently computes the full pipeline
for its 48 attention rows and the host reassembles along W.

Layout: padded row stride 104 (8 zero cols serve as both right halo of
row r and left halo of row r+1). bf16 inputs/weights with fp32 PSUM
accumulation; Phase-B weights preloaded during Phase A. Phase A is
branch-interleaved and software-pipelined (attention tail of group g
emitted under group g+1's convs) so the in-order PE queue never blocks
on the ACT/DVE softmax chain.
"""
import numpy as np
import ml_dtypes

import concourse.bass as bass
import concourse.mybir as mybir
import concourse.tile as tile
from concourse import bacc
from concourse.bass_utils import run_bass_kernel_spmd
from concourse.masks import make_identity

F32 = mybir.dt.float32
F32R = mybir.dt.float32r
BF16 = mybir.dt.bfloat16
BF = ml_dtypes.bfloat16

B, HH, WW, C = 4, 96, 96, 384
HD = 128
SCALE = float((HD // 3) ** -0.5)  # 42**-0.5
N_CORES = 8
R = 48            # attention rows per core
RP = 66           # slack row + 8 halo + 48 + 8 halo + 1 slack row
SP = 104          # padded row stride (8 zero pad + 96 valid)
T = R * 96        # tokens per core = 4608
GROUPS = R // 4   # 12 groups of 4 rows
GN = 4 * 96       # attention tokens per group = 384
CN = 4 * SP       # conv tokens per group (with pads) = 416
# MLP tiles: 8x512 then 2x256 (narrow tail shortens the end-of-kernel drain)
TILES = [(t * 512, 512) for t in range(8)] + [(4096, 256), (4352, 256)]

TAPS_A = [
    [(dr, ds) for dr in (-1, 0, 1) for ds in (-1, 0, 1)],
    [(dr, 0) for dr in range(-4, 5)],
    [(0, ds) for ds in range(-4, 5)],
]
TAPS_B = [
    [(dr, ds) for dr in (-2, 0, 2) for ds in (-2, 0, 2)],
    [(dr, 0) for dr in range(-8, 9, 2)],
    [(0, ds) for ds in range(-8, 9, 2)],
]

_CACHED_NC = None


def build_kernel():
    nc = bacc.Bacc("TRN2", target_bir_lowering=False, debug=False)

    xp = nc.dram_tensor("xp", [C, RP, SP], BF16, kind="ExternalInput").ap()
    wca = nc.dram_tensor("wca", [3, HD, 9, HD], BF16, kind="ExternalInput").ap()
    wcb = nc.dram_tensor("wcb", [3, HD, 9, HD], BF16, kind="ExternalInput").ap()
    bca = nc.dram_tensor("bca", [HD, 3], F32, kind="ExternalInput").ap()
    bcb = nc.dram_tensor("bcb", [HD, 3], F32, kind="ExternalInput").ap()
    wgd = nc.dram_tensor("wgd", [3, HD, HD], BF16, kind="ExternalInput").ap()
    eb3 = nc.dram_tensor("eb3", [3, 384], BF16, kind="ExternalInput").ap()
    wqk = nc.dram_tensor("wqk", [3, HD, 256], BF16, kind="ExternalInput").ap()
    wv = nc.dram_tensor("wv", [3, HD, HD], BF16, kind="ExternalInput").ap()
    bap = nc.dram_tensor("bap", [HD, 3], F32, kind="ExternalInput").ap()
    wgf = nc.dram_tensor("wgf", [3, HD, HD], BF16, kind="ExternalInput").ap()
    w1 = nc.dram_tensor("w1", [3, HD, 3, 1536], BF16, kind="ExternalInput").ap()
    b1 = nc.dram_tensor("b1", [HD, 3, 12], F32, kind="ExternalInput").ap()
    w2 = nc.dram_tensor("w2", [3, HD, 12, C], BF16, kind="ExternalInput").ap()
    b2r = nc.dram_tensor("b2r", [3, C], BF16, kind="ExternalInput").ap()
    bpr = nc.dram_tensor("bpr", [HD, 3], F32, kind="ExternalInput").ap()
    out_cm = nc.dram_tensor("out_cm", [C, T], F32, kind="ExternalOutput").ap()

    with tile.TileContext(nc) as tc:
        with tc.tile_pool(name="consts", bufs=1) as consts, \
             tc.tile_pool(name="persist", bufs=1) as persist:
            ident = consts.tile([HD, HD], F32)
            make_identity(nc, ident)
            identb = consts.tile([HD, HD], BF16)
            nc.vector.tensor_copy(identb, ident)

            bca_sb = persist.tile([HD, 3], F32)
            nc.sync.dma_start(out=bca_sb, in_=bca)
            bcb_sb = persist.tile([HD, 3], F32)
            nc.sync.dma_start(out=bcb_sb, in_=bcb)
            bap_sb = persist.tile([HD, 3], F32)
            nc.sync.dma_start(out=bap_sb, in_=bap)

            xc_t = [persist.tile([HD, T], BF16, tag=f"xc{i}", name=f"xc{i}") for i in range(3)]

            # Phase-B weights: preloaded during Phase A (contiguous DMA).
            b1_sb = persist.tile([HD, 3, 12], F32)
            b2r_sb = persist.tile([3, C], BF16)
            wgf_sb = persist.tile([HD, 3, HD], BF16)
            bpr_sb = persist.tile([HD, 3], F32)
            eb3_sb = persist.tile([3, 384], BF16)
            w1_sb = [persist.tile([HD, 3, 1536], BF16, tag=f"w1_{e}", name=f"w1_{e}")
                     for e in range(3)]
            w2_sb = [persist.tile([HD, 12, C], BF16, tag=f"w2_{e}", name=f"w2_{e}")
                     for e in range(3)]

            # ---------------- Phase A: conv MoE + attention, interleaved --
            with tc.tile_pool(name="xw", bufs=1) as xw, \
                 tc.tile_pool(name="gpool", bufs=3) as gpool, \
                 tc.tile_pool(name="apool", bufs=3) as apool, \
                 tc.tile_pool(name="psC", bufs=3, space="PSUM") as psC, \
                 tc.tile_pool(name="psT", bufs=5, space="PSUM") as psT:
                xp_sb, wgd_sb, wca_sb, wcb_sb, wqk_sb, wv_sb = \
                    [], [], [], [], [], []
                for i in range(3):
                    xp_sb.append(xw.tile([HD, RP, SP], BF16, tag=f"xp{i}",
                                         name=f"xp{i}"))
                    nc.sync.dma_start(out=xp_sb[i][:, :21, :],
                                      in_=xp[i * HD:(i + 1) * HD, :21, :])
                    t_ = xw.tile([HD, HD], BF16, tag=f"wgd{i}", name=f"wgd{i}")
                    nc.sync.dma_start(out=t_, in_=wgd[i])
                    wgd_sb.append(t_)
                    t_ = xw.tile([HD, 9, HD], BF16, tag=f"wca{i}", name=f"wca{i}")
                    nc.sync.dma_start(out=t_, in_=wca[i])
                    wca_sb.append(t_)
                    t_ = xw.tile([HD, 9, HD], BF16, tag=f"wcb{i}", name=f"wcb{i}")
                    nc.sync.dma_start(out=t_, in_=wcb[i])
                    wcb_sb.append(t_)
                    t_ = xw.tile([HD, 256], BF16, tag=f"wqk{i}", name=f"wqk{i}")
                    nc.sync.dma_start(out=t_, in_=wqk[i])
                    wqk_sb.append(t_)
                    t_ = xw.tile([HD, HD], BF16, tag=f"wv{i}", name=f"wv{i}")
                    nc.sync.dma_start(out=t_, in_=wv[i])
                    wv_sb.append(t_)
                for i in range(3):
                    nc.sync.dma_start(out=xp_sb[i][:, 21:43, :],
                                      in_=xp[i * HD:(i + 1) * HD, 21:43, :])
                for i in range(3):
                    nc.sync.dma_start(out=xp_sb[i][:, 43:, :],
                                      in_=xp[i * HD:(i + 1) * HD, 43:, :])
                # Phase-B weights stream in the background
                nc.sync.dma_start(out=b1_sb, in_=b1)
                nc.sync.dma_start(out=b2r_sb, in_=b2r)
                nc.sync.dma_start(out=wgf_sb, in_=wgf.rearrange("a p b -> p a b"))
                nc.sync.dma_start(out=bpr_sb, in_=bpr)
                nc.sync.dma_start(out=eb3_sb, in_=eb3)
                for e in range(3):
                    nc.sync.dma_start(out=w1_sb[e], in_=w1[e])
                    nc.sync.dma_start(out=w2_sb[e], in_=w2[e])

                xpf = [xp_sb[i].rearrange("p r s -> p (r s)") for i in range(3)]

                def stage1(i, g):
                    """gate + both expert convs + moe blend for (branch, group)."""
                    fo = (9 + 4 * g) * SP
                    plg = psC.tile([HD, CN], F32, tag="ps")
                    nc.tensor.matmul(plg, wgd_sb[i], xpf[i][:, fo:fo + CN],
                                     start=True, stop=True)
                    ex = gpool.tile([HD, CN], F32, tag="ex")
                    nc.scalar.activation(ex, plg,
                                         mybir.ActivationFunctionType.Tanh,
                                         scale=-0.5)
                    pa = psC.tile([HD, CN], F32, tag="ps")
                    for ti, (dr, ds) in enumerate(TAPS_A[i]):
                        nc.tensor.matmul(
                            pa, wca_sb[i][:, ti, :],
                            xpf[i][:, fo + dr * SP + ds: fo + dr * SP + ds + CN],
                            start=(ti == 0), stop=(ti == 8))
                    pb = psC.tile([HD, CN], F32, tag="ps")
                    for ti, (dr, ds) in enumerate(TAPS_B[i]):
                        nc.tensor.matmul(
                            pb, wcb_sb[i][:, ti, :],
                            xpf[i][:, fo + dr * SP + ds: fo + dr * SP + ds + CN],
                            start=(ti == 0), stop=(ti == 8))
                    # moe = g0*(ca - cb) + cb  (bias-add fused on ACT)
                    ca = gpool.tile([HD, CN], F32, tag="ca")
                    nc.scalar.activation(ca, pa,
                                         mybir.ActivationFunctionType.Identity,
                                         bias=bca_sb[:, i:i + 1], scale=0.5)
                    cb = gpool.tile([HD, CN], F32, tag="cb")
                    nc.scalar.activation(cb, pb,
                                         mybir.ActivationFunctionType.Identity,
                                         bias=bcb_sb[:, i:i + 1], scale=0.5)
                    dd = gpool.tile([HD, CN], F32, tag="dd")
                    nc.vector.tensor_sub(dd, ca, cb)
                    d2 = gpool.tile([HD, CN], F32, tag="d2")
                    nc.vector.tensor_mul(d2, dd, ex)
                    ss = gpool.tile([HD, CN], F32, tag="ss")
                    nc.vector.tensor_add(ss, ca, cb)
                    moe = gpool.tile([HD, CN], BF16, tag="moe")
                    nc.vector.tensor_add(moe, ss, d2)
                    return moe

                def stage2(i, moe):
                    """q/k/v projections."""
                    pq = psT.tile([HD, CN], F32, tag="ps")
                    nc.tensor.matmul(pq, wqk_sb[i][:, 0:HD], moe,
                                     start=True, stop=True)
                    q_sb = gpool.tile([HD, CN], BF16, tag="q")
                    nc.scalar.copy(q_sb, pq)
                    pk = psT.tile([HD, CN], F32, tag="ps")
                    nc.tensor.matmul(pk, wqk_sb[i][:, HD:256], moe,
                                     start=True, stop=True)
                    k_sb = gpool.tile([HD, CN], BF16, tag="k")
                    nc.scalar.copy(k_sb, pk)
                    pvt = psT.tile([96, 4 * HD], F32, tag="ps")
                    for j in range(4):
                        nc.tensor.matmul(pvt[:, j * HD:(j + 1) * HD],
                                         moe[:, j * SP + 8: j * SP + SP],
                                         wv_sb[i], start=True, stop=True)
                    vt_sb = apool.tile([96, 4 * HD], BF16, tag="vt")
                    nc.vector.tensor_copy(vt_sb, pvt)
                    return q_sb, k_sb, vt_sb

                def stage3(q_sb, k_sb):
                    """scores + softmax numerator/denominator."""
                    psc = psT.tile([96, GN], F32, tag="ps")
                    for j in range(4):
                        nc.tensor.matmul(psc[:, j * 96:(j + 1) * 96],
                                         q_sb[:, j * SP + 8: j * SP + SP],
                                         k_sb[:, j * SP + 8: j * SP + SP],
                                         start=True, stop=True)
                    probs = apool.tile([96, GN], BF16, tag="probs")
                    nc.scalar.activation(probs, psc,
                                         mybir.ActivationFunctionType.Exp,
                                         scale=SCALE)
                    zsum = apool.tile([96, 4], F32, tag="zsum")
                    nc.vector.tensor_reduce(
                        zsum, probs.rearrange("p (j q) -> p j q", q=96),
                        axis=mybir.AxisListType.X, op=mybir.AluOpType.add)
                    rec = apool.tile([96, 4], F32, tag="rec")
                    nc.vector.reciprocal(rec, zsum)
                    pn = apool.tile([96, GN], BF16, tag="pn")
                    for j in range(4):
                        nc.scalar.activation(
                            pn[:, j * 96:(j + 1) * 96],
                            probs[:, j * 96:(j + 1) * 96],
                            mybir.ActivationFunctionType.Copy,
                            scale=rec[:, j:j + 1])
                    return pn

                def stage4(i, g, pn, vt_sb):
                    """probs transpose + attention output + xc write."""
                    ppt = psT.tile([96, GN], BF16, tag="ps")
                    for j in range(4):
                        nc.tensor.transpose(ppt[:, j * 96:(j + 1) * 96],
                                            pn[:, j * 96:(j + 1) * 96],
                                            identb[:96, :96])
                    pt_sb = apool.tile([96, GN], BF16, tag="pt")
                    nc.vector.tensor_copy(pt_sb, ppt)
                    po = psT.tile([HD, GN], F32, tag="ps")
                    for j in range(4):
                        nc.tensor.matmul(po[:, j * 96:(j + 1) * 96],
                                         vt_sb[:, j * HD:(j + 1) * HD],
                                         pt_sb[:, j * 96:(j + 1) * 96],
                                         start=True, stop=True)
                    nc.scalar.activation(
                        xc_t[i][:, g * GN:(g + 1) * GN], po,
                        mybir.ActivationFunctionType.Identity,
                        bias=bap_sb[:, i:i + 1])

                prev = None  # [(pn, vt), ...] per branch for group g-1
                for g in range(GROUPS):
                    moes = []
                    for i in range(3):
                        moes.append(stage1(i, g))
                        if i == 0 and prev is not None:
                            for ii in range(3):
                                stage4(ii, g - 1, *prev[ii])
                    qkv = [stage2(i, moes[i]) for i in range(3)]
                    prev = [(stage3(qkv[i][0], qkv[i][1]), qkv[i][2])
                            for i in range(3)]
                for ii in range(3):
                    stage4(ii, GROUPS - 1, *prev[ii])

            # ---------------- Phase B: final MLP MoE + proj ---------------
            with tc.tile_pool(name="bpool", bufs=3) as bpool, \
                 tc.tile_pool(name="gpoolB", bufs=2) as gpoolB, \
                 tc.tile_pool(name="psL", bufs=3, space="PSUM") as psL, \
                 tc.tile_pool(name="psGB", bufs=1, space="PSUM") as psGB, \
                 tc.tile_pool(name="psPG", bufs=1, space="PSUM") as psPG, \
                 tc.tile_pool(name="psB", bufs=3, space="PSUM") as psB:

                def gating_part1a(t):
                    """logits matmul for tile t."""
                    t0, nt = TILES[t]
                    plg = psGB.tile([HD, 512], F32, tag="ps", name="plg")
                    for kc in range(3):
                        nc.tensor.matmul(plg[:, :nt], wgf_sb[:, kc, :],
                                         xc_t[kc][:, t0:t0 + nt],
                                         start=(kc == 0), stop=(kc == 2))
                    lsb = gpoolB.tile([3, 512], F32, tag="lsb", name="lsb")
                    nc.vector.tensor_copy(lsb[:, :nt], plg[0:3, :nt])
                    return lsb

                def gating_part1b(lsb, t):
                    """token-major top-2 softmax math."""
                    nt = TILES[t][1]
                    n4 = nt // HD  # 4 or 2 column-blocks of 128 tokens
                    plt = psGB.tile([HD, 12], F32, tag="ps", name="plt")
                    for t4 in range(n4):
                        nc.tensor.transpose(plt[:, t4 * 3:(t4 + 1) * 3],
                                            lsb[:, t4 * HD:(t4 + 1) * HD],
                                            ident[:3, :3])
                    lt = gpoolB.tile([HD, 12], F32, tag="lt", name="lt")
                    nc.vector.tensor_copy(lt[:, :3 * n4], plt[:, :3 * n4])
                    l3 = lt.rearrange("p (j e) -> p j e", e=3)
                    mx = gpoolB.tile([HD, 4], F32, tag="mx", name="mx")
                    nc.vector.tensor_reduce(mx[:, :n4], l3[:, :n4],
                                            axis=mybir.AxisListType.X,
                                            op=mybir.AluOpType.max)
                    mn = gpoolB.tile([HD, 4], F32, tag="mn", name="mn")
                    nc.vector.tensor_reduce(mn[:, :n4], l3[:, :n4],
                                            axis=mybir.AxisListType.X,
                                            op=mybir.AluOpType.min)
                    sm = gpoolB.tile([HD, 4], F32, tag="sm", name="sm")
                    nc.vector.tensor_reduce(sm[:, :n4], l3[:, :n4],
                                            axis=mybir.AxisListType.X,
                                            op=mybir.AluOpType.add)
                    t1 = gpoolB.tile([HD, 4], F32, tag="t1", name="t1")
                    nc.vector.tensor_sub(t1[:, :n4], sm[:, :n4], mx[:, :n4])
                    mid = gpoolB.tile([HD, 4], F32, tag="mid", name="mid")
                    nc.vector.tensor_sub(mid[:, :n4], t1[:, :n4], mn[:, :n4])
                    dm = gpoolB.tile([HD, 4], F32, tag="dm", name="dm")
                    nc.vector.tensor_sub(dm[:, :n4], mx[:, :n4], mid[:, :n4])
                    th = gpoolB.tile([HD, 4], F32, tag="th", name="th")
                    nc.scalar.activation(th[:, :n4], dm[:, :n4],
                                         mybir.ActivationFunctionType.Tanh,
                                         scale=0.5)
                    gmx = gpoolB.tile([HD, 4], F32, tag="gmx", name="gmx")
                    nc.vector.tensor_scalar(gmx[:, :n4], th[:, :n4], 0.5, 0.5,
                                            op0=mybir.AluOpType.mult,
                                            op1=mybir.AluOpType.add)
                    eqx = gpoolB.tile([HD, 12], F32, tag="eqx", name="eqx")
                    eqn = gpoolB.tile([HD, 12], F32, tag="eqn", name="eqn")
                    for t4 in range(n4):
                        sl = slice(t4 * 3, (t4 + 1) * 3)
                        nc.vector.tensor_scalar(eqx[:, sl], lt[:, sl],
                                                mx[:, t4:t4 + 1], None,
                                                op0=mybir.AluOpType.is_equal)
                        nc.vector.tensor_scalar(eqn[:, sl], lt[:, sl],
                                                mn[:, t4:t4 + 1], None,
                                                op0=mybir.AluOpType.is_equal)
                    # u = 1 - eqx - eqn (mid indicator); g = gmx*(eqx-u) + u
                    s1 = gpoolB.tile([HD, 12], F32, tag="s1", name="s1")
                    nc.vector.tensor_add(s1[:, :3 * n4], eqx[:, :3 * n4],
                                         eqn[:, :3 * n4])
                    u = gpoolB.tile([HD, 12], F32, tag="u", name="u")
                    nc.vector.tensor_scalar(u[:, :3 * n4], s1[:, :3 * n4],
                                            -1.0, 1.0,
                                            op0=mybir.AluOpType.mult,
                                            op1=mybir.AluOpType.add)
                    d0 = gpoolB.tile([HD, 12], F32, tag="d0", name="d0")
                    nc.vector.tensor_sub(d0[:, :3 * n4], eqx[:, :3 * n4],
                                         u[:, :3 * n4])
                    p0 = gpoolB.tile([HD, 12], F32, tag="p0", name="p0")
                    for t4 in range(n4):
                        sl = slice(t4 * 3, (t4 + 1) * 3)
                        nc.vector.tensor_scalar_mul(p0[:, sl], d0[:, sl],
                                                    gmx[:, t4:t4 + 1])
                    gm = gpoolB.tile([HD, 12], BF16, tag="gm", name="gm")
                    nc.vector.tensor_add(gm[:, :3 * n4], p0[:, :3 * n4],
                                         u[:, :3 * n4])
                    return gm

                def gating_part2(gm, t):
                    """expert-major gates [3, nt] from token-major gm."""
                    nt = TILES[t][1]
                    n4 = nt // HD
                    pgt = psGB.tile([3, 512], BF16, tag="ps", name="pgt")
                    for t4 in range(n4):
                        nc.tensor.transpose(pgt[:, t4 * HD:(t4 + 1) * HD],
                                            gm[:, t4 * 3:(t4 + 1) * 3],
                                            identb)
                    gates_r = gpoolB.tile([3, 512], BF16, tag="gates",
                                          name="gates_r")
                    nc.scalar.copy(gates_r[:, :nt], pgt[:, :nt])
                    return gates_r

                gm_next = gating_part1b(gating_part1a(0), 0)
                for t in range(len(TILES)):
                    t0, nt = TILES[t]
                    gates_r = gating_part2(gm_next, t)
                    lsb_next = gating_part1a(t + 1) if t + 1 < len(TILES) else None

                    pd = [psL.tile([HD, 512], F32, tag="down", name=f"pd{_i}") for _i in range(3)]
                    for e in range(3):
                        if e == 1 and lsb_next is not None:
                            gm_next = gating_part1b(lsb_next, t + 1)
                        pgb = psPG.tile([HD, 512], F32, tag="pgb", name="pgb")
                        nc.tensor.matmul(pgb[:, :nt], eb3_sb[:, e * HD:(e + 1) * HD],
                                         gates_r[:, :nt], start=True, stop=True)
                        for m in range(12):
                            pu = psB.tile([HD, 512], F32, tag="ps", name="pu")
                            for kc in range(3):
                                nc.tensor.matmul(
                                    pu[:, :nt], w1_sb[e][:, kc, m * HD:(m + 1) * HD],
                                    xc_t[kc][:, t0:t0 + nt],
                                    start=(kc == 0), stop=(kc == 2))
                            h = bpool.tile([HD, 512], F32, tag="h")
                            nc.scalar.activation(
                                h[:, :nt], pu[:, :nt],
                                mybir.ActivationFunctionType.Gelu,
                                bias=b1_sb[:, e, m:m + 1])
                            hs = bpool.tile([HD, 512], BF16, tag="hs")
                            nc.vector.tensor_mul(hs[:, :nt], h[:, :nt],
                                                 pgb[:, :nt])
                            for mp in range(3):
                                nc.tensor.matmul(
                                    pd[mp][:, :nt],
                                    w2_sb[e][:, m, mp * HD:(mp + 1) * HD],
                                    hs[:, :nt], start=(e == 0 and m == 0),
                                    stop=False)
                    for mp in range(3):
                        nc.tensor.matmul(pd[mp][:, :nt],
                                         b2r_sb[:, mp * HD:(mp + 1) * HD],
                                         gates_r[:, :nt], start=False, stop=True)
                    for mp in range(3):
                        osb = bpool.tile([HD, 512], F32, tag="osb")
                        nc.scalar.activation(osb[:, :nt], pd[mp][:, :nt],
                                             mybir.ActivationFunctionType.Identity,
                                             bias=bpr_sb[:, mp:mp + 1])
                        nc.sync.dma_start(
                            out=out_cm[mp * HD:(mp + 1) * HD, t0:t0 + nt],
                            in_=osb[:, :nt])
    nc.compile()
    return nc


def _prep_inputs(x, w_e1, b_e1, w_e2, b_e2, w_e3, b_e3, w_e4, b_e4, w_e5, b_e5,
                 w_e6, b_e6, wg1, wg2, wg3, w_qkv, w_attn_proj, b_attn_proj,
                 wg_final, w_mlp1, b_mlp1, w_mlp2, b_mlp2, w_proj, b_proj):
    f = np.float32
    shared = {}
    # conv weights pre-transposed to [cin(p), tap, cout] for contiguous DMA
    shared["wca"] = np.ascontiguousarray(np.stack([
        w_e1.reshape(9, HD, HD).transpose(1, 0, 2),
        w_e3.reshape(9, HD, HD).transpose(1, 0, 2),
        w_e5.reshape(9, HD, HD).transpose(1, 0, 2)]).astype(BF))
    shared["wcb"] = np.ascontiguousarray(np.stack([
        w_e2.reshape(9, HD, HD).transpose(1, 0, 2),
        w_e4.reshape(9, HD, HD).transpose(1, 0, 2),
        w_e6.reshape(9, HD, HD).transpose(1, 0, 2)]).astype(BF))
    shared["bca"] = np.ascontiguousarray(
        np.stack([b_e1, b_e3, b_e5], axis=1) * 0.5, dtype=f)
    shared["bcb"] = np.ascontiguousarray(
        np.stack([b_e2, b_e4, b_e6], axis=1) * 0.5, dtype=f)
    wgs = np.stack([wg1, wg2, wg3])
    shared["wgd"] = np.ascontiguousarray(
        np.repeat((wgs[:, :, 1] - wgs[:, :, 0])[:, :, None], HD, axis=2)
        .astype(BF))
    eb3 = np.zeros((3, 384), f)
    for e in range(3):
        eb3[e, e * 128:(e + 1) * 128] = 1.0
    shared["eb3"] = eb3.astype(BF)
    shared["wqk"] = np.ascontiguousarray(np.asarray(w_qkv[:, :, :256]).astype(BF))
    wv64 = np.asarray(w_qkv[:, :, 256:], dtype=np.float64)
    wap64 = np.asarray(w_attn_proj, dtype=np.float64)
    shared["wv"] = np.ascontiguousarray(
        np.einsum("ick,iko->ico", wv64, wap64).astype(BF))
    shared["bap"] = np.ascontiguousarray(b_attn_proj.T, dtype=f)
    shared["wgf"] = np.ascontiguousarray(
        np.tile(wg_final.reshape(3, HD, 3), (1, 1, 43))[:, :, :HD].astype(BF))
    shared["w1"] = np.ascontiguousarray(
        w_mlp1.reshape(3, 3, HD, 1536).transpose(0, 2, 1, 3).astype(BF))
    shared["b1"] = np.ascontiguousarray(
        b_mlp1.reshape(3, 12, HD).transpose(2, 0, 1), dtype=f)
    w2p = np.asarray(w_mlp2, dtype=np.float64) @ np.asarray(w_proj, np.float64)
    shared["w2"] = np.ascontiguousarray(
        w2p.reshape(3, 12, HD, C).transpose(0, 2, 1, 3).astype(BF))
    shared["b2r"] = np.ascontiguousarray(
        (np.asarray(b_mlp2, np.float64) @ np.asarray(w_proj, np.float64))
        .astype(BF))
    shared["bpr"] = np.ascontiguousarray(b_proj.reshape(3, HD).T, dtype=f)

    in_maps = []
    for c in range(N_CORES):
        b, half = c // 2, c % 2
        r0 = half * R
        slab = np.zeros((C, RP, SP), BF)
        glo, ghi = max(0, r0 - 8), min(HH, r0 + R + 8)
        plo = glo - (r0 - 8) + 1
        slab[:, plo:plo + (ghi - glo), 8:SP] = \
            np.asarray(x[b, glo:ghi]).astype(BF).transpose(2, 0, 1)
        m = dict(shared)
        m["xp"] = np.ascontiguousarray(slab)
        in_maps.append(m)
    return in_maps


def kernel(**inputs):
    global _CACHED_NC
    if _CACHED_NC is None:
        _CACHED_NC = build_kernel()
    nc = _CACHED_NC
    in_maps = _prep_inputs(**{k: np.asarray(v) for k, v in inputs.items()})
    res = None
    for attempt in range(3):
        try:
            res = run_bass_kernel_spmd(nc, in_maps,
                                       core_ids=list(range(N_CORES)))
            break
        except Exception:
            if attempt == 2:
                raise
            import time
            time.sleep(2.0)
    out = np.empty((B, HH, WW, C), np.float32)
    for c in range(N_CORES):
        b, half = c // 2, c % 2
        slab = res.results[c]["out_cm"].reshape(C, R, 96)
        out[b, :, half * R:(half + 1) * R, :] = slab.transpose(2, 1, 0)
    return out


# revision 5
# speedup vs baseline: 1.1935x; 1.0801x over previous
"""Trainium2 Bass kernel for nn_MAMoE (conv-MoE -> row attention -> MLP-MoE).

Sharding: 8 cores = (batch b in 0..3) x (H-half in 0..1). All routing is
per-token; the reference's swapaxes(1,2) means attention row r produces
output column w=r, so each core independently computes the full pipeline
for its 48 attention rows and the host reassembles along W.

Layout: padded row stride 104 (8 zero cols serve as both right halo of
row r and left halo of row r+1); conv/gate matmuls use strided rhs APs
([4 rows @ 104, 96]) so no pad columns are ever computed. scores use a
host-fused A = scale * Wq @ Wk^T so only one projection (qh) is needed.
bf16 everywhere with fp32 PSUM accumulation. Phase A is branch-interleaved
and software-pipelined two groups deep (scores of group g-1 and attention
tail of group g-2 are emitted under group g's convs) so the in-order PE
queue never blocks on the ACT/DVE softmax chain. Phase-B weights preload
on the second hardware DMA queue during Phase A.
"""
import numpy as np
import ml_dtypes

import concourse.bass as bass
import concourse.mybir as mybir
import concourse.tile as tile
from concourse import bacc
from concourse.bass_utils import run_bass_kernel_spmd
from concourse.masks import make_identity

F32 = mybir.dt.float32
F32R = mybir.dt.float32r
BF16 = mybir.dt.bfloat16
BF = ml_dtypes.bfloat16

B, HH, WW, C = 4, 96, 96, 384
HD = 128
SCALE = float((HD // 3) ** -0.5)  # 42**-0.5
N_CORES = 8
R = 48            # attention rows per core
RP = 66           # slack row + 8 halo + 48 + 8 halo + 1 slack row
SP = 104          # padded row stride (8 zero pad + 96 valid)
T = R * 96        # tokens per core = 4608
GROUPS = R // 4   # 12 groups of 4 rows
GN = 4 * 96       # tokens per group = 384
# MLP tiles: 8x512 then 2x256 (narrow tail shortens the end-of-kernel drain)
TILES = [(t * 512, 512) for t in range(8)] + [(4096, 256), (4352, 256)]

TAPS_A = [
    [(dr, ds) for dr in (-1, 0, 1) for ds in (-1, 0, 1)],
    [(dr, 0) for dr in range(-4, 5)],
    [(0, ds) for ds in range(-4, 5)],
]
TAPS_B = [
    [(dr, ds) for dr in (-2, 0, 2) for ds in (-2, 0, 2)],
    [(dr, 0) for dr in range(-8, 9, 2)],
    [(0, ds) for ds in range(-8, 9, 2)],
]

_CACHED_NC = None


def build_kernel():
    nc = bacc.Bacc("TRN2", target_bir_lowering=False, debug=False)

    xp = nc.dram_tensor("xp", [C, RP, SP], BF16, kind="ExternalInput").ap()
    wca = nc.dram_tensor("wca", [3, HD, 9, HD], BF16, kind="ExternalInput").ap()
    wcb = nc.dram_tensor("wcb", [3, HD, 9, HD], BF16, kind="ExternalInput").ap()
    bca = nc.dram_tensor("bca", [HD, 3], F32, kind="ExternalInput").ap()
    bcb = nc.dram_tensor("bcb", [HD, 3], F32, kind="ExternalInput").ap()
    wgd = nc.dram_tensor("wgd", [3, HD, HD], BF16, kind="ExternalInput").ap()
    eb3 = nc.dram_tensor("eb3", [3, 384], BF16, kind="ExternalInput").ap()
    wqh = nc.dram_tensor("wqh", [3, HD, HD], BF16, kind="ExternalInput").ap()
    wv = nc.dram_tensor("wv", [3, HD, HD], BF16, kind="ExternalInput").ap()
    bap = nc.dram_tensor("bap", [HD, 3], F32, kind="ExternalInput").ap()
    wgf = nc.dram_tensor("wgf", [3, HD, HD], BF16, kind="ExternalInput").ap()
    w1 = nc.dram_tensor("w1", [3, HD, 3, 1536], BF16, kind="ExternalInput").ap()
    b1 = nc.dram_tensor("b1", [HD, 3, 12], F32, kind="ExternalInput").ap()
    w2 = nc.dram_tensor("w2", [3, HD, 12, C], BF16, kind="ExternalInput").ap()
    b2r = nc.dram_tensor("b2r", [3, C], BF16, kind="ExternalInput").ap()
    bpr = nc.dram_tensor("bpr", [HD, 3], F32, kind="ExternalInput").ap()
    out_cm = nc.dram_tensor("out_cm", [C, T], F32, kind="ExternalOutput").ap()

    with tile.TileContext(nc) as tc:
        with tc.tile_pool(name="consts", bufs=1) as consts, \
             tc.tile_pool(name="persist", bufs=1) as persist:
            ident = consts.tile([HD, HD], F32)
            make_identity(nc, ident)
            identb = consts.tile([HD, HD], BF16)
            nc.vector.tensor_copy(identb, ident)

            bca_sb = persist.tile([HD, 3], F32)
            bcb_sb = persist.tile([HD, 3], F32)
            bap_sb = persist.tile([HD, 3], F32)

            xc_t = [persist.tile([HD, T], BF16, tag=f"xc{i}", name=f"xc{i}") for i in range(3)]

            # Phase-B weights (DMAs issued later, on the scalar HWDGE queue)
            b1_sb = persist.tile([HD, 3, 12], F32)
            b2r_sb = persist.tile([3, C], BF16)
            wgf_sb = persist.tile([HD, 3, HD], BF16)
            bpr_sb = persist.tile([HD, 3], F32)
            eb3_sb = persist.tile([3, 384], BF16)
            w1_sb = [persist.tile([HD, 3, 1536], BF16, tag=f"w1_{e}", name=f"w1_{e}")
                     for e in range(3)]
            w2_sb = [persist.tile([HD, 12, C], BF16, tag=f"w2_{e}", name=f"w2_{e}")
                     for e in range(3)]

            phase_b_loads = []
            for e in range(3):
                phase_b_loads.append((w1_sb[e], w1[e]))
                phase_b_loads.append((w2_sb[e], w2[e]))
            phase_b_loads += [
                (b1_sb, b1), (b2r_sb, b2r), (bpr_sb, bpr), (eb3_sb, eb3),
            ]

            # ---------------- Phase A: conv MoE + attention, interleaved --
            with tc.tile_pool(name="xw", bufs=1) as xw, \
                 tc.tile_pool(name="gp3", bufs=3) as gp3, \
                 tc.tile_pool(name="gp6", bufs=6) as gp6, \
                 tc.tile_pool(name="ap3", bufs=3) as ap3, \
                 tc.tile_pool(name="ap9", bufs=9) as ap9, \
                 tc.tile_pool(name="psC", bufs=3, space="PSUM") as psC, \
                 tc.tile_pool(name="psT", bufs=5, space="PSUM") as psT:
                xp_sb = [xw.tile([HD, RP, SP], BF16, tag=f"xp{i}", name=f"xp{i}")
                         for i in range(3)]
                wgd_sb, wca_sb, wcb_sb, wqh_sb, wv_sb = [], [], [], [], []
                for i in range(3):
                    wgd_sb.append(xw.tile([HD, HD], BF16, tag=f"wgd{i}", name=f"wgd{i}"))
                    wca_sb.append(xw.tile([HD, 9, HD], BF16, tag=f"wca{i}", name=f"wca{i}"))
                    wcb_sb.append(xw.tile([HD, 9, HD], BF16, tag=f"wcb{i}", name=f"wcb{i}"))
                    wqh_sb.append(xw.tile([HD, HD], BF16, tag=f"wqh{i}", name=f"wqh{i}"))
                    wv_sb.append(xw.tile([HD, HD], BF16, tag=f"wv{i}", name=f"wv{i}"))

                def loadw(i, eng):
                    eng.dma_start(out=wgd_sb[i], in_=wgd[i])
                    eng.dma_start(out=wca_sb[i], in_=wca[i])
                    eng.dma_start(out=wcb_sb[i], in_=wcb[i])
                    eng.dma_start(out=wqh_sb[i], in_=wqh[i])
                    eng.dma_start(out=wv_sb[i], in_=wv[i])

                # Criticality-ordered DMA issue across both HWDGE queues.
                nc.sync.dma_start(out=xp_sb[0][:, :25, :], in_=xp[0:HD, :25, :])
                loadw(0, nc.sync)
                nc.scalar.dma_start(out=xp_sb[1][:, :25, :],
                                    in_=xp[HD:2 * HD, :25, :])
                nc.scalar.dma_start(out=xp_sb[2][:, :25, :],
                                    in_=xp[2 * HD:3 * HD, :25, :])
                nc.scalar.dma_start(out=bca_sb, in_=bca)
                nc.scalar.dma_start(out=bcb_sb, in_=bcb)
                nc.scalar.dma_start(out=bap_sb, in_=bap)
                loadw(1, nc.sync)
                loadw(2, nc.sync)
                for i in range(3):
                    nc.sync.dma_start(out=xp_sb[i][:, 25:45, :],
                                      in_=xp[i * HD:(i + 1) * HD, 25:45, :])
                for i in range(3):
                    nc.sync.dma_start(out=xp_sb[i][:, 45:, :],
                                      in_=xp[i * HD:(i + 1) * HD, 45:, :])
                nc.sync.dma_start(out=wgf_sb, in_=wgf.rearrange("a p b -> p a b"))

                xpf = [xp_sb[i].rearrange("p r s -> p (r s)") for i in range(3)]

                def win(i, g, dr, ds):
                    """[128, 4, 96] strided window: rows (9+4g+dr).., col 8+ds."""
                    base = (9 + 4 * g + dr) * SP + 8 + ds
                    return xpf[i][:, base:base + 4 * SP] \
                        .rearrange("p (r s) -> p r s", s=SP)[:, :, :96]

                def stage1(i, g):
                    """gate + both expert convs + moe blend for (branch, group)."""
                    plg = psC.tile([HD, GN], F32, tag="ps")
                    nc.tensor.matmul(plg, wgd_sb[i], win(i, g, 0, 0),
                                     start=True, stop=True)
                    ex = gp3.tile([HD, GN], BF16, tag="ex")
                    nc.scalar.activation(ex, plg,
                                         mybir.ActivationFunctionType.Tanh,
                                         scale=-0.5)
                    pa = psC.tile([HD, GN], F32, tag="ps")
                    for ti, (dr, ds) in enumerate(TAPS_A[i]):
                        nc.tensor.matmul(pa, wca_sb[i][:, ti, :], win(i, g, dr, ds),
                                         start=(ti == 0), stop=(ti == 8))
                    pb = psC.tile([HD, GN], F32, tag="ps")
                    for ti, (dr, ds) in enumerate(TAPS_B[i]):
                        nc.tensor.matmul(pb, wcb_sb[i][:, ti, :], win(i, g, dr, ds),
                                         start=(ti == 0), stop=(ti == 8))
                    # moe = g0*(ca - cb) + cb  (bias-add fused on ACT)
                    ca = gp3.tile([HD, GN], BF16, tag="ca")
                    nc.scalar.activation(ca, pa,
                                         mybir.ActivationFunctionType.Identity,
                                         bias=bca_sb[:, i:i + 1], scale=0.5)
                    cb = gp3.tile([HD, GN], BF16, tag="cb")
                    nc.scalar.activation(cb, pb,
                                         mybir.ActivationFunctionType.Identity,
                                         bias=bcb_sb[:, i:i + 1], scale=0.5)
                    dd = gp3.tile([HD, GN], BF16, tag="dd")
                    nc.vector.tensor_sub(dd, ca, cb)
                    d2 = gp3.tile([HD, GN], BF16, tag="d2")
                    nc.vector.tensor_mul(d2, dd, ex)
                    ss = gp3.tile([HD, GN], BF16, tag="ss")
                    nc.vector.tensor_add(ss, ca, cb)
                    moe = gp6.tile([HD, GN], BF16, tag="moe")
                    nc.vector.tensor_add(moe, ss, d2)
                    return moe

                def stage2(i, moe):
                    """fused qh = (scale*Wq@Wk^T)^T moe and v (w/ proj fused)."""
                    pqh = psT.tile([HD, GN], F32, tag="ps")
                    nc.tensor.matmul(pqh, wqh_sb[i], moe, start=True, stop=True)
                    qh = gp6.tile([HD, GN], BF16, tag="qh")
                    nc.scalar.copy(qh, pqh)
                    pvt = psT.tile([96, 4 * HD], F32, tag="ps")
                    for j in range(4):
                        nc.tensor.matmul(pvt[:, j * HD:(j + 1) * HD],
                                         moe[:, j * 96:(j + 1) * 96],
                                         wv_sb[i], start=True, stop=True)
                    vt_sb = ap9.tile([96, 4 * HD], BF16, tag="vt")
                    nc.vector.tensor_copy(vt_sb, pvt)
                    return qh, vt_sb

                def stage3(qh, moe):
                    """scores + softmax numerator/denominator."""
                    psc = psT.tile([96, GN], F32, tag="ps")
                    for j in range(4):
                        nc.tensor.matmul(psc[:, j * 96:(j + 1) * 96],
                                         qh[:, j * 96:(j + 1) * 96],
                                         moe[:, j * 96:(j + 1) * 96],
                                         start=True, stop=True)
                    probs = ap3.tile([96, GN], BF16, tag="probs")
                    nc.scalar.activation(probs, psc,
                                         mybir.ActivationFunctionType.Exp)
                    zsum = ap3.tile([96, 4], F32, tag="zsum")
                    nc.vector.tensor_reduce(
                        zsum, probs.rearrange("p (j q) -> p j q", q=96),
                        axis=mybir.AxisListType.X, op=mybir.AluOpType.add)
                    rec = ap3.tile([96, 4], F32, tag="rec")
                    nc.vector.reciprocal(rec, zsum)
                    pn = gp6.tile([96, GN], BF16, tag="pn")
                    for j in range(4):
                        nc.vector.tensor_scalar_mul(
                            pn[:, j * 96:(j + 1) * 96],
                            probs[:, j * 96:(j + 1) * 96], rec[:, j:j + 1])
                    return pn

                def stage4(i, g, pn, vt_sb):
                    """probs transpose + attention output + xc write."""
                    ppt = psT.tile([96, GN], BF16, tag="ps")
                    for j in range(4):
                        nc.tensor.transpose(ppt[:, j * 96:(j + 1) * 96],
                                            pn[:, j * 96:(j + 1) * 96],
                                            identb[:96, :96])
                    pt_sb = ap3.tile([96, GN], BF16, tag="pt")
                    nc.vector.tensor_copy(pt_sb, ppt)
                    po = psT.tile([HD, GN], F32, tag="ps")
                    for j in range(4):
                        nc.tensor.matmul(po[:, j * 96:(j + 1) * 96],
                                         vt_sb[:, j * HD:(j + 1) * HD],
                                         pt_sb[:, j * 96:(j + 1) * 96],
                                         start=True, stop=True)
                    nc.vector.tensor_scalar(
                        xc_t[i][:, g * GN:(g + 1) * GN], po,
                        bap_sb[:, i:i + 1], None, op0=mybir.AluOpType.add)

                s2 = {}  # g -> [(qh, vt), ...]; smoe: g -> [moe, ...]
                smoe = {}
                s3 = {}  # g -> [pn, ...]
                for g in range(GROUPS):
                    moes = []
                    for i in range(3):
                        moes.append(stage1(i, g))
                        if i == 0 and g - 1 in s2:
                            s3[g - 1] = [stage3(s2[g - 1][ii][0],
                                                smoe[g - 1][ii])
                                         for ii in range(3)]
                        if i == 1 and g - 2 in s3:
                            for ii in range(3):
                                stage4(ii, g - 2, s3[g - 2][ii],
                                       s2[g - 2][ii][1])
                            del s3[g - 2], s2[g - 2], smoe[g - 2]
                    # stream Phase-B weights on the scalar queue mid-phase
                    if 2 <= g <= 6:
                        for dst, src in phase_b_loads[2 * (g - 2):2 * (g - 1)]:
                            nc.scalar.dma_start(out=dst, in_=src)
                    s2[g] = [stage2(i, moes[i]) for i in range(3)]
                    smoe[g] = moes
                g = GROUPS
                s3[g - 1] = [stage3(s2[g - 1][ii][0], smoe[g - 1][ii])
                             for ii in range(3)]
                for gg in (g - 2, g - 1):
                    for ii in range(3):
                        stage4(ii, gg, s3[gg][ii], s2[gg][ii][1])

            # ---------------- Phase B: final MLP MoE + proj ---------------
            with tc.tile_pool(name="bpool", bufs=3) as bpool, \
                 tc.tile_pool(name="gpoolB", bufs=2) as gpoolB, \
                 tc.tile_pool(name="psL", bufs=3, space="PSUM") as psL, \
                 tc.tile_pool(name="psGB", bufs=1, space="PSUM") as psGB, \
                 tc.tile_pool(name="psPG", bufs=1, space="PSUM") as psPG, \
                 tc.tile_pool(name="psB", bufs=3, space="PSUM") as psB:

                def gating_part1a(t):
                    """logits matmul for tile t."""
                    t0, nt = TILES[t]
                    plg = psGB.tile([HD, 512], F32, tag="ps", name="plg")
                    for kc in range(3):
                        nc.tensor.matmul(plg[:, :nt], wgf_sb[:, kc, :],
                                         xc_t[kc][:, t0:t0 + nt],
                                         start=(kc == 0), stop=(kc == 2))
                    lsb = gpoolB.tile([3, 512], F32, tag="lsb", name="lsb")
                    nc.vector.tensor_copy(lsb[:, :nt], plg[0:3, :nt])
                    return lsb

                def gating_part1b(lsb, t):
                    """token-major top-2 softmax math."""
                    nt = TILES[t][1]
                    n4 = nt // HD  # 4 or 2 column-blocks of 128 tokens
                    plt = psGB.tile([HD, 12], F32, tag="ps", name="plt")
                    for t4 in range(n4):
                        nc.tensor.transpose(plt[:, t4 * 3:(t4 + 1) * 3],
                                            lsb[:, t4 * HD:(t4 + 1) * HD],
                                            ident[:3, :3])
                    lt = gpoolB.tile([HD, 12], F32, tag="lt", name="lt")
                    nc.vector.tensor_copy(lt[:, :3 * n4], plt[:, :3 * n4])
                    l3 = lt.rearrange("p (j e) -> p j e", e=3)
                    mx = gpoolB.tile([HD, 4], F32, tag="mx", name="mx")
                    nc.vector.tensor_reduce(mx[:, :n4], l3[:, :n4],
                                            axis=mybir.AxisListType.X,
                                            op=mybir.AluOpType.max)
                    mn = gpoolB.tile([HD, 4], F32, tag="mn", name="mn")
                    nc.vector.tensor_reduce(mn[:, :n4], l3[:, :n4],
                                            axis=mybir.AxisListType.X,
                                            op=mybir.AluOpType.min)
                    sm = gpoolB.tile([HD, 4], F32, tag="sm", name="sm")
                    nc.vector.tensor_reduce(sm[:, :n4], l3[:, :n4],
                                            axis=mybir.AxisListType.X,
                                            op=mybir.AluOpType.add)
                    t1 = gpoolB.tile([HD, 4], F32, tag="t1", name="t1")
                    nc.vector.tensor_sub(t1[:, :n4], sm[:, :n4], mx[:, :n4])
                    mid = gpoolB.tile([HD, 4], F32, tag="mid", name="mid")
                    nc.vector.tensor_sub(mid[:, :n4], t1[:, :n4], mn[:, :n4])
                    dm = gpoolB.tile([HD, 4], F32, tag="dm", name="dm")
                    nc.vector.tensor_sub(dm[:, :n4], mx[:, :n4], mid[:, :n4])
                    th = gpoolB.tile([HD, 4], F32, tag="th", name="th")
                    nc.scalar.activation(th[:, :n4], dm[:, :n4],
                                         mybir.ActivationFunctionType.Tanh,
                                         scale=0.5)
                    gmx = gpoolB.tile([HD, 4], F32, tag="gmx", name="gmx")
                    nc.vector.tensor_scalar(gmx[:, :n4], th[:, :n4], 0.5, 0.5,
                                            op0=mybir.AluOpType.mult,
                                            op1=mybir.AluOpType.add)
                    eqx = gpoolB.tile([HD, 12], F32, tag="eqx", name="eqx")
                    eqn = gpoolB.tile([HD, 12], F32, tag="eqn", name="eqn")
                    for t4 in range(n4):
                        sl = slice(t4 * 3, (t4 + 1) * 3)
                        nc.vector.tensor_scalar(eqx[:, sl], lt[:, sl],
                                                mx[:, t4:t4 + 1], None,
                                                op0=mybir.AluOpType.is_equal)
                        nc.vector.tensor_scalar(eqn[:, sl], lt[:, sl],
                                                mn[:, t4:t4 + 1], None,
                                                op0=mybir.AluOpType.is_equal)
                    # u = 1 - eqx - eqn (mid indicator); g = gmx*(eqx-u) + u
                    s1 = gpoolB.tile([HD, 12], F32, tag="s1", name="s1")
                    nc.vector.tensor_add(s1[:, :3 * n4], eqx[:, :3 * n4],
                                         eqn[:, :3 * n4])
                    u = gpoolB.tile([HD, 12], F32, tag="u", name="u")
                    nc.vector.tensor_scalar(u[:, :3 * n4], s1[:, :3 * n4],
                                            -1.0, 1.0,
                                            op0=mybir.AluOpType.mult,
                                            op1=mybir.AluOpType.add)
                    d0 = gpoolB.tile([HD, 12], F32, tag="d0", name="d0")
                    nc.vector.tensor_sub(d0[:, :3 * n4], eqx[:, :3 * n4],
                                         u[:, :3 * n4])
                    p0 = gpoolB.tile([HD, 12], F32, tag="p0", name="p0")
                    for t4 in range(n4):
                        sl = slice(t4 * 3, (t4 + 1) * 3)
                        nc.vector.tensor_scalar_mul(p0[:, sl], d0[:, sl],
                                                    gmx[:, t4:t4 + 1])
                    gm = gpoolB.tile([HD, 12], BF16, tag="gm", name="gm")
                    nc.vector.tensor_add(gm[:, :3 * n4], p0[:, :3 * n4],
                                         u[:, :3 * n4])
                    return gm

                def gating_part2(gm, t):
                    """expert-major gates [3, nt] from token-major gm."""
                    nt = TILES[t][1]
                    n4 = nt // HD
                    pgt = psGB.tile([3, 512], BF16, tag="ps", name="pgt")
                    for t4 in range(n4):
                        nc.tensor.transpose(pgt[:, t4 * HD:(t4 + 1) * HD],
                                            gm[:, t4 * 3:(t4 + 1) * 3],
                                            identb)
                    gates_r = gpoolB.tile([3, 512], BF16, tag="gates",
                                          name="gates_r")
                    nc.scalar.copy(gates_r[:, :nt], pgt[:, :nt])
                    return gates_r

                gm_next = gating_part1b(gating_part1a(0), 0)
                for t in range(len(TILES)):
                    t0, nt = TILES[t]
                    gates_r = gating_part2(gm_next, t)
                    lsb_next = gating_part1a(t + 1) if t + 1 < len(TILES) else None

                    pd = [psL.tile([HD, 512], F32, tag="down", name=f"pd{_i}") for _i in range(3)]
                    for e in range(3):
                        if e == 1 and lsb_next is not None:
                            gm_next = gating_part1b(lsb_next, t + 1)
                        pgb = psPG.tile([HD, 512], F32, tag="pgb", name="pgb")
                        nc.tensor.matmul(pgb[:, :nt], eb3_sb[:, e * HD:(e + 1) * HD],
                                         gates_r[:, :nt], start=True, stop=True)
                        for m in range(12):
                            pu = psB.tile([HD, 512], F32, tag="ps", name="pu")
                            for kc in range(3):
                                nc.tensor.matmul(
                                    pu[:, :nt], w1_sb[e][:, kc, m * HD:(m + 1) * HD],
                                    xc_t[kc][:, t0:t0 + nt],
                                    start=(kc == 0), stop=(kc == 2))
                            h = bpool.tile([HD, 512], F32, tag="h")
                            nc.scalar.activation(
                                h[:, :nt], pu[:, :nt],
                                mybir.ActivationFunctionType.Gelu,
                                bias=b1_sb[:, e, m:m + 1])
                            hs = bpool.tile([HD, 512], BF16, tag="hs")
                            nc.vector.tensor_mul(hs[:, :nt], h[:, :nt],
                                                 pgb[:, :nt])
                            for mp in range(3):
                                nc.tensor.matmul(
                                    pd[mp][:, :nt],
                                    w2_sb[e][:, m, mp * HD:(mp + 1) * HD],
                                    hs[:, :nt], start=(e == 0 and m == 0),
                                    stop=False)
                    for mp in range(3):
                        nc.tensor.matmul(pd[mp][:, :nt],
                                         b2r_sb[:, mp * HD:(mp + 1) * HD],
                                         gates_r[:, :nt], start=False, stop=True)
                    for mp in range(3):
                        osb = bpool.tile([HD, 512], F32, tag="osb")
                        nc.scalar.activation(osb[:, :nt], pd[mp][:, :nt],
                                             mybir.ActivationFunctionType.Identity,
                                             bias=bpr_sb[:, mp:mp + 1])
                        nc.sync.dma_start(
                            out=out_cm[mp * HD:(mp + 1) * HD, t0:t0 + nt],
                            in_=osb[:, :nt])
    nc.compile()
    return nc


def _prep_inputs(x, w_e1, b_e1, w_e2, b_e2, w_e3, b_e3, w_e4, b_e4, w_e5, b_e5,
                 w_e6, b_e6, wg1, wg2, wg3, w_qkv, w_attn_proj, b_attn_proj,
                 wg_final, w_mlp1, b_mlp1, w_mlp2, b_mlp2, w_proj, b_proj):
    f = np.float32
    shared = {}
    # conv weights pre-transposed to [cin(p), tap, cout] for contiguous DMA
    shared["wca"] = np.ascontiguousarray(np.stack([
        w_e1.reshape(9, HD, HD).transpose(1, 0, 2),
        w_e3.reshape(9, HD, HD).transpose(1, 0, 2),
        w_e5.reshape(9, HD, HD).transpose(1, 0, 2)]).astype(BF))
    shared["wcb"] = np.ascontiguousarray(np.stack([
        w_e2.reshape(9, HD, HD).transpose(1, 0, 2),
        w_e4.reshape(9, HD, HD).transpose(1, 0, 2),
        w_e6.reshape(9, HD, HD).transpose(1, 0, 2)]).astype(BF))
    shared["bca"] = np.ascontiguousarray(
        np.stack([b_e1, b_e3, b_e5], axis=1) * 0.5, dtype=f)
    shared["bcb"] = np.ascontiguousarray(
        np.stack([b_e2, b_e4, b_e6], axis=1) * 0.5, dtype=f)
    wgs = np.stack([wg1, wg2, wg3])
    shared["wgd"] = np.ascontiguousarray(
        np.repeat((wgs[:, :, 1] - wgs[:, :, 0])[:, :, None], HD, axis=2)
        .astype(BF))
    eb3 = np.zeros((3, 384), f)
    for e in range(3):
        eb3[e, e * 128:(e + 1) * 128] = 1.0
    shared["eb3"] = eb3.astype(BF)
    # fused score matrix A = SCALE * Wq @ Wk^T  (scores = moe A moe^T)
    wq64 = np.asarray(w_qkv[:, :, :HD], dtype=np.float64)
    wk64 = np.asarray(w_qkv[:, :, HD:2 * HD], dtype=np.float64)
    shared["wqh"] = np.ascontiguousarray(
        (SCALE * np.einsum("iac,ibc->iab", wq64, wk64)).astype(BF))
    wv64 = np.asarray(w_qkv[:, :, 2 * HD:], dtype=np.float64)
    wap64 = np.asarray(w_attn_proj, dtype=np.float64)
    shared["wv"] = np.ascontiguousarray(
        np.einsum("ick,iko->ico", wv64, wap64).astype(BF))
    shared["bap"] = np.ascontiguousarray(b_attn_proj.T, dtype=f)
    shared["wgf"] = np.ascontiguousarray(
        np.tile(wg_final.reshape(3, HD, 3), (1, 1, 43))[:, :, :HD].astype(BF))
    shared["w1"] = np.ascontiguousarray(
        w_mlp1.reshape(3, 3, HD, 1536).transpose(0, 2, 1, 3).astype(BF))
    shared["b1"] = np.ascontiguousarray(
        b_mlp1.reshape(3, 12, HD).transpose(2, 0, 1), dtype=f)
    w2p = np.asarray(w_mlp2, dtype=np.float64) @ np.asarray(w_proj, np.float64)
    shared["w2"] = np.ascontiguousarray(
        w2p.reshape(3, 12, HD, C).transpose(0, 2, 1, 3).astype(BF))
    shared["b2r"] = np.ascontiguousarray(
        (np.asarray(b_mlp2, np.float64) @ np.asarray(w_proj, np.float64))
        .astype(BF))
    shared["bpr"] = np.ascontiguousarray(b_proj.reshape(3, HD).T, dtype=f)

    in_maps = []
    for c in range(N_CORES):
        b, half = c // 2, c % 2
        r0 = half * R
        slab = np.zeros((C, RP, SP), BF)
        glo, ghi = max(0, r0 - 8), min(HH, r0 + R + 8)
        plo = glo - (r0 - 8) + 1
        slab[:, plo:plo + (ghi - glo), 8:SP] = \
            np.asarray(x[b, glo:ghi]).astype(BF).transpose(2, 0, 1)
        m = dict(shared)
        m["xp"] = np.ascontiguousarray(slab)
        in_maps.append(m)
    return in_maps


def kernel(**inputs):
    global _CACHED_NC
    if _CACHED_NC is None:
        _CACHED_NC = build_kernel()
    nc = _CACHED_NC
    in_maps = _prep_inputs(**{k: np.asarray(v) for k, v in inputs.items()})
    res = None
    for attempt in range(3):
        try:
            res = run_bass_kernel_spmd(nc, in_maps,
                                       core_ids=list(range(N_CORES)))
            break
        except Exception:
            if attempt == 2:
                raise
            import time
            time.sleep(2.0)
    out = np.empty((B, HH, WW, C), np.float32)
    for c in range(N_CORES):
        b, half = c // 2, c % 2
        slab = res.results[c]["out_cm"].reshape(C, R, 96)
        out[b, :, half * R:(half + 1) * R, :] = slab.transpose(2, 1, 0)
    return out
